# revision 1
# baseline (speedup 1.0000x reference)
"""HOPE block kernel for 8 Trainium2 NeuronCores.

Sharding: 8 shards = (batch b in 0..3, sequence half in 0..1), 2048 tokens each.
The linear-attention memory scan is causal per (batch, head); cores owning the
second half of a sequence receive the first half as a "prefix" input and
rebuild the mid-sequence memory state M (= sum_t k_t v_t^T per head) before
scanning their own chunks. Even cores receive a zero prefix, keeping the SPMD
program uniform.

v2: bf16 operands for every matmul (fp32r pays 4 cycles/row below N=256, bf16
is 1 everywhere; weights/activations halve DMA + SBUF), the CMS FFN keeps h /
gelu activations / the partial-sum accumulator resident in SBUF (no DRAM
round-trips; only w1/w2 stream, once per level), persistent tile pools so
blocks/levels pipeline, and the scan loops chunk-outer/head-inner so the
per-head memory update chain is 16 iterations deep instead of 1.
"""
import sys
if '/opt/trn_rl_repo' not in sys.path:
    sys.path.insert(0, '/opt/trn_rl_repo')

from contextlib import ExitStack
import numpy as np


def _ensure_ntff_hook():
    """Register the axon NTFF profile hook when the image's antenv stub
    lacks `axon_hooks` — otherwise BASS_TRACE runs skip tracing and
    exec_time_ns comes back None. No-op when the real module exists."""
    import types, contextlib, ctypes, os
    try:
        from antenv.axon_hooks import get_axon_ntff_profile_hook  # noqa: F401
        return
    except ImportError:
        pass
    hook = None
    so_path = '/opt/axon/libaxon_pjrt.so'
    if os.path.exists(so_path):
        try:
            lib = ctypes.CDLL(so_path)
        except OSError:
            lib = None
        if lib is not None and hasattr(lib, 'axon_start_nrt_profile'):
            lib.axon_start_nrt_profile.argtypes = [
                ctypes.POINTER(ctypes.c_int64), ctypes.c_size_t]
            lib.axon_start_nrt_profile.restype = ctypes.c_int64
            lib.axon_stop_nrt_profile.argtypes = [ctypes.c_char_p]
            lib.axon_stop_nrt_profile.restype = ctypes.c_int64

            @contextlib.contextmanager
            def hook(output_dir, device_ids):
                import jax
                jax.devices()  # force PJRT init so the .so's client exists
                if device_ids:
                    ids = (ctypes.c_int64 * len(device_ids))(*device_ids)
                    rc = lib.axon_start_nrt_profile(ids, len(device_ids))
                else:
                    rc = lib.axon_start_nrt_profile(None, 0)
                if rc != 0:
                    raise RuntimeError(f"axon_start_nrt_profile rc={rc}")
                try:
                    yield
                finally:
                    n = lib.axon_stop_nrt_profile(str(output_dir).encode())
                    print(f"profile: {n} file(s) written to {output_dir}",
                          file=sys.stderr)

    try:
        import antenv
    except ImportError:
        return
    mod = types.ModuleType('antenv.axon_hooks')
    _h = hook
    mod.get_axon_ntff_profile_hook = lambda: _h
    mod.set_axon_ntff_profile_hook = lambda h: None
    sys.modules['antenv.axon_hooks'] = mod
    antenv.axon_hooks = mod


_ensure_ntff_hook()

import concourse.bass as bass
import concourse.tile as tile
from concourse import mybir
from concourse.bass_utils import run_bass_kernel_spmd
from concourse.masks import make_identity

f32 = mybir.dt.float32
bf16 = mybir.dt.bfloat16
f8 = mybir.dt.float8e4
AF = mybir.ActivationFunctionType
ALU = mybir.AluOpType
PM = mybir.MatmulPerfMode

W8_SCALE = 64.0     # host-side multiplier on fp8 FFN weights (clears the
INV_W8 = 1.0 / W8_SCALE  # e4m3 subnormal zone); un-done via activation scale

DIM = 1024
HEADS = 16
HD = 64
B, S = 4, 4096
LEVELS = 3
HID = 4 * DIM
CHUNK = 128
EPS = 1e-5
P = 128

N_CORES = 8
T_OWN = S // 2      # tokens per core
T_PRE = S // 2      # prefix tokens (zeros on even cores)
BLK = 512           # token block for phases A and B
D_T = DIM // P      # 8 feature tiles
H_T = HID // P      # 32 hidden tiles
HHT = H_T // 2      # 16 hidden tiles per half
TT = 512            # FFN token tile

MAX_WAITS = 1


def _split_multi_waits(nc, max_waits=MAX_WAITS):
    """Walrus in this toolchain encodes at most `max_waits` sem waits per
    instruction; split extra waits onto same-engine NOPs placed just before."""
    for f in nc.m.functions:
        for bb in f.blocks:
            insts = list(bb.instructions)
            if not any(
                i.sync_info and i.sync_info.on_wait and len(i.sync_info.on_wait) > max_waits
                for i in insts
            ):
                continue
            new = []
            for inst in insts:
                si = inst.sync_info
                waits = list(si.on_wait) if si and si.on_wait else []
                if len(waits) > max_waits:
                    head, rest = waits[:-max_waits], waits[-max_waits:]
                    while head:
                        chunk, head = head[:max_waits], head[max_waits:]
                        nop = mybir.InstNoOp(name=nc.get_next_instruction_name(), ins=[], outs=[])
                        nop.engine = inst.engine
                        nop.sync_info = mybir.SyncInfo(on_wait=chunk, on_update=[])
                        nc.register_instruction(nop, overwrite=True)
                        new.append(nop)
                    inst.sync_info = mybir.SyncInfo(
                        on_wait=rest, on_update=list(si.on_update) if si.on_update else [])
                new.append(inst)
            bb.instructions = new


def _layernorm_tile(nc, pools, x_t, g_bc, b_bc, eps_t, out_r):
    """LayerNorm of one [128, DIM] fp32 tile -> bf16 tile (token-major)."""
    w = pools
    BNF = nc.vector.BN_STATS_FMAX
    nsub = DIM // BNF
    stats = w.tile([P, nsub, nc.vector.BN_STATS_DIM], f32, tag="ln_stats")
    xg = x_t[:].rearrange("p (s f) -> p s f", f=BNF)
    for s_ in range(nsub):
        nc.vector.bn_stats(out=stats[:, s_, :], in_=xg[:, s_, :])
    mv = w.tile([P, nc.vector.BN_AGGR_DIM], f32, tag="ln_mv")
    nc.vector.bn_aggr(out=mv, in_=stats)
    rstd = w.tile([P, 1], f32, tag="ln_rstd")
    nc.scalar.activation(out=rstd, in_=mv[:, 1:2], func=AF.Sqrt, bias=eps_t, scale=1.0)
    nc.vector.reciprocal(out=rstd, in_=rstd)
    tmp = w.tile([P, DIM], f32, tag="ln_tmp")
    nc.vector.tensor_scalar(out=tmp, in0=x_t, scalar1=mv[:, 0:1], scalar2=rstd,
                            op0=ALU.subtract, op1=ALU.mult)
    nc.vector.tensor_mul(out=tmp, in0=tmp, in1=g_bc)
    nc.vector.tensor_add(out=out_r, in0=tmp, in1=b_bc)


def build_kernel(t_own=T_OWN, t_pre=T_PRE):
    nc = bass.Bass()

    x_own = nc.dram_tensor("x_own", [t_own, DIM], f32, kind="ExternalInput")
    x_pre = nc.dram_tensor("x_pre", [t_pre, DIM], f32, kind="ExternalInput")
    wq = nc.dram_tensor("wq", [DIM, DIM], bf16, kind="ExternalInput")
    wk = nc.dram_tensor("wk", [DIM, DIM], bf16, kind="ExternalInput")
    wv = nc.dram_tensor("wv", [DIM, DIM], bf16, kind="ExternalInput")
    wo = nc.dram_tensor("wo", [DIM, DIM], bf16, kind="ExternalInput")
    ln1_g = nc.dram_tensor("ln1_g", [DIM], f32, kind="ExternalInput")
    ln1_b = nc.dram_tensor("ln1_b", [DIM], f32, kind="ExternalInput")
    ln2_g = nc.dram_tensor("ln2_g", [DIM], f32, kind="ExternalInput")
    ln2_b = nc.dram_tensor("ln2_b", [DIM], f32, kind="ExternalInput")
    cms_w1 = nc.dram_tensor("cms_w1", [LEVELS, DIM, HID], f8, kind="ExternalInput")
    cms_b1 = nc.dram_tensor("cms_b1", [LEVELS, HID], f32, kind="ExternalInput")
    cms_w2 = nc.dram_tensor("cms_w2", [LEVELS, HID, DIM], f8, kind="ExternalInput")
    cms_b2 = nc.dram_tensor("cms_b2", [LEVELS, DIM], f32, kind="ExternalInput")
    maskT = nc.dram_tensor("maskT", [CHUNK, CHUNK], f32, kind="ExternalInput")
    out = nc.dram_tensor("out", [t_own, DIM], f32, kind="ExternalOutput")

    n_own_t = t_own // P           # 16 token tiles of 128
    n_blk = t_own // BLK           # 4 blocks
    n_pre_blk = t_pre // BLK       # 4 prefix blocks
    ntt = BLK // P                 # 4 token tiles per block
    n_tt = t_own // TT             # 4 FFN token tiles

    with tile.TileContext(nc) as tc, ExitStack() as top:
        consts = top.enter_context(tc.tile_pool(name="consts", bufs=1))
        ident_f = consts.tile([P, P], f32)
        make_identity(nc, ident_f)
        ident = consts.tile([P, P], bf16)
        nc.vector.tensor_copy(out=ident, in_=ident_f)
        eps_t = consts.tile([P, 1], f32)
        nc.vector.memset(eps_t, EPS)
        mask_t = consts.tile([CHUNK, CHUNK], f32)
        nc.sync.dma_start(out=mask_t, in_=maskT.ap())
        # [e, c0|c1] mask for super-chunk scores: left = causal, right = all-1
        # (chunk c1 attends to every token of chunk c0)
        mask2 = consts.tile([CHUNK, 2 * CHUNK], f32)
        nc.sync.dma_start(out=mask2[:, 0:CHUNK], in_=maskT.ap())
        nc.vector.memset(mask2[:, CHUNK:2 * CHUNK], 1.0)
        g1 = consts.tile([P, DIM], f32)
        b1 = consts.tile([P, DIM], f32)
        g2 = consts.tile([P, DIM], f32)
        b2 = consts.tile([P, DIM], f32)
        nc.sync.dma_start(out=g1, in_=ln1_g.ap()[None, :].partition_broadcast(P).opt())
        nc.sync.dma_start(out=b1, in_=ln1_b.ap()[None, :].partition_broadcast(P).opt())
        nc.sync.dma_start(out=g2, in_=ln2_g.ap()[None, :].partition_broadcast(P).opt())
        nc.sync.dma_start(out=b2, in_=ln2_b.ap()[None, :].partition_broadcast(P).opt())

        # persistent activation: h2^T (feature-major, FFN input/output),
        # fp8 so it feeds DoubleRow matmuls; x2 residual spills to DRAM bf16
        persist = top.enter_context(tc.tile_pool(name="persist", bufs=1))
        hT = persist.tile([P, D_T, t_own], f8)
        dram = top.enter_context(tc.tile_pool(name="dram", bufs=1, space="DRAM"))
        x2_d = dram.tile([n_own_t, P, DIM], bf16)

        # ---------------- attention phases ----------------
        ab_stack = ExitStack()
        wo_pool = ab_stack.enter_context(tc.tile_pool(name="wo_pool", bufs=1))
        wo_sb = wo_pool.tile([P, D_T, DIM], bf16)
        wo_all = wo.ap().rearrange("(kt p) d -> p kt d", p=P)
        nc.scalar.dma_start(out=wo_sb, in_=wo_all)

        mt_pool = ab_stack.enter_context(tc.tile_pool(name="mt", bufs=1))
        Mt_f = mt_pool.tile([P, HEADS // 2, HD], f32)   # head h at [pb:pb+64, h//2]
        Mt_s = mt_pool.tile([P, HEADS // 2, HD], bf16)
        nc.vector.memset(Mt_f, 0.0)

        ln_w = ab_stack.enter_context(tc.tile_pool(name="ln_w", bufs=2))
        xp = ab_stack.enter_context(tc.tile_pool(name="xp", bufs=2))
        hrp = ab_stack.enter_context(tc.tile_pool(name="hrp", bufs=2))
        h1Tp = ab_stack.enter_context(tc.tile_pool(name="h1Tp", bufs=2))
        wsp = ab_stack.enter_context(tc.tile_pool(name="wsp", bufs=2))
        actp = ab_stack.enter_context(tc.tile_pool(name="actp", bufs=1))
        ps_tp = ab_stack.enter_context(tc.tile_pool(name="ps_tp", bufs=2, space="PSUM"))
        ps_mm = ab_stack.enter_context(tc.tile_pool(name="ps_mm", bufs=2, space="PSUM"))
        ps_scan = ab_stack.enter_context(tc.tile_pool(name="ps_scan", bufs=2, space="PSUM"))

        def ln_transpose_block(x_src, tok0, g_bc, b_bc, dstT):
            """DMA 4 x-tiles, LayerNorm, PE-transpose into dstT [P, D_T, BLK].
            Returns the list of x tiles (fp32) for residual use."""
            xts = []
            for t in range(ntt):
                x_t = xp.tile([P, DIM], f32, tag=f"x{t % 2}")
                nc.sync.dma_start(out=x_t, in_=x_src.ap()[tok0 + t * P:tok0 + (t + 1) * P, :])
                h_r = hrp.tile([P, DIM], bf16, tag=f"h{t % 2}")
                _layernorm_tile(nc, ln_w, x_t, g_bc, b_bc, eps_t, h_r)
                for fidx in range(D_T):
                    tps = ps_tp.tile([P, P], bf16, tag="tp_ps")
                    nc.tensor.transpose(tps, h_r[:, fidx * P:(fidx + 1) * P], ident)
                    nc.scalar.copy(out=dstT[:, fidx, t * P:(t + 1) * P], in_=tps)
                xts.append(x_t)
            return xts

        # ---------------- Phase A: prefix -> Mt ----------------
        for blk in range(n_pre_blk):
            hpT = h1Tp.tile([P, D_T, BLK], bf16, tag="h1T")
            ln_transpose_block(x_pre, blk * BLK, g1, b1, hpT)
            kc = actp.tile([P, ntt, DIM], bf16, tag="kc")
            vc = actp.tile([P, ntt, DIM], bf16, tag="vc", bufs=2)
            for (w_in, dst) in ((wk, kc), (wv, vc)):
                w_all = w_in.ap().rearrange("(kt p) d -> p kt d", p=P)
                for nh in range(2):
                    w_t = wsp.tile([P, D_T, 512], bf16, tag="w_t")
                    nc.sync.dma_start(out=w_t, in_=w_all[:, :, nh * 512:(nh + 1) * 512])
                    for m in range(ntt):
                        pst = ps_mm.tile([P, 512], f32, tag="pst")
                        for k in range(D_T):
                            nc.tensor.matmul(pst, hpT[:, k, m * P:(m + 1) * P], w_t[:, k, :],
                                             start=(k == 0), stop=(k == D_T - 1))
                        nc.scalar.copy(out=dst[:, m, nh * 512:(nh + 1) * 512], in_=pst)
            # Mt += kc^T vc per head, accumulated in PSUM across the block
            for h in range(HEADS):
                pb = (h % 2) * HD
                mt_ps = ps_scan.tile([HD, HD], f32, tag="kc" if h % 2 == 0 else "mt")
                for ch in range(ntt):
                    nc.tensor.matmul(mt_ps, kc[:, ch, h * HD:(h + 1) * HD],
                                     vc[:, ch, h * HD:(h + 1) * HD],
                                     start=(ch == 0), stop=(ch == ntt - 1))
                nc.vector.tensor_add(out=Mt_f[pb:pb + HD, h // 2, :],
                                     in0=Mt_f[pb:pb + HD, h // 2, :], in1=mt_ps)
        nc.scalar.copy(out=Mt_s, in_=Mt_f)

        # ---------------- Phase B: own tokens, attention ----------------
        scw = ab_stack.enter_context(tc.tile_pool(name="scw", bufs=3))
        x2fp = ab_stack.enter_context(tc.tile_pool(name="x2fp", bufs=2))
        yTp = ab_stack.enter_context(tc.tile_pool(name="yTp", bufs=2))
        for blk in range(n_blk):
            tok0 = blk * BLK
            h1T = h1Tp.tile([P, D_T, BLK], bf16, tag="h1T")
            ln_transpose_block(x_own, tok0, g1, b1, h1T)
            qT = actp.tile([P, D_T, BLK], bf16, tag="qT", bufs=2)
            kT = actp.tile([P, D_T, BLK], bf16, tag="kT", bufs=2)
            for (w_in, dst) in ((wq, qT), (wk, kT)):
                w_all = w_in.ap().rearrange("(kt p) d -> p kt d", p=P)
                for nh in range(2):
                    w_t = wsp.tile([P, D_T, 512], bf16, tag="w_t")
                    nc.sync.dma_start(out=w_t, in_=w_all[:, :, nh * 512:(nh + 1) * 512])
                    for ml in range(4):
                        m = nh * 4 + ml
                        pst = ps_mm.tile([P, BLK], f32, tag="pst")
                        for k in range(D_T):
                            nc.tensor.matmul(pst, w_t[:, k, ml * P:(ml + 1) * P], h1T[:, k, :],
                                             start=(k == 0), stop=(k == D_T - 1))
                        nc.scalar.copy(out=dst[:, m, :], in_=pst)
            v = actp.tile([P, ntt, DIM], bf16, tag="vc", bufs=2)
            wv_all = wv.ap().rearrange("(kt p) d -> p kt d", p=P)
            for nh in range(2):
                w_t = wsp.tile([P, D_T, 512], bf16, tag="w_t")
                nc.sync.dma_start(out=w_t, in_=wv_all[:, :, nh * 512:(nh + 1) * 512])
                for m in range(ntt):
                    pst = ps_mm.tile([P, 512], f32, tag="pst")
                    for k in range(D_T):
                        nc.tensor.matmul(pst, h1T[:, k, m * P:(m + 1) * P], w_t[:, k, :],
                                         start=(k == 0), stop=(k == D_T - 1))
                    nc.scalar.copy(out=v[:, m, nh * 512:(nh + 1) * 512], in_=pst)
            # scan over super-chunks of 256 tokens: the c0->c1 cross block is
            # the unmasked right half of one [128,256] score matmul, so Mt is
            # read/updated once per 256 tokens (halves the serial chain)
            y = actp.tile([P, ntt, DIM], bf16, tag="y")
            for sch in range(ntt // 2):
                c0 = 2 * sch
                c1 = c0 + 1
                for h in range(HEADS):
                    pb = (h % 2) * HD
                    fi = h // 2
                    q01 = qT[pb:pb + HD, fi, c0 * P:(c0 + 2) * P]
                    q0 = qT[pb:pb + HD, fi, c0 * P:(c0 + 1) * P]
                    q1 = qT[pb:pb + HD, fi, c1 * P:(c1 + 1) * P]
                    k0 = kT[pb:pb + HD, fi, c0 * P:(c0 + 1) * P]
                    k1 = kT[pb:pb + HD, fi, c1 * P:(c1 + 1) * P]
                    v0 = v[:, c0, h * HD:(h + 1) * HD]
                    v1 = v[:, c1, h * HD:(h + 1) * HD]
                    kc_ps = ps_scan.tile([P, 2, HD], bf16, tag="kc")
                    nc.tensor.transpose(kc_ps[:, 0, :], k0, ident[pb:pb + HD, pb:pb + HD])
                    nc.tensor.transpose(kc_ps[:, 1, :], k1, ident[pb:pb + HD, pb:pb + HD])
                    kc_s = scw.tile([P, 2, HD], bf16, tag="kc_s")
                    nc.scalar.copy(out=kc_s, in_=kc_ps)
                    sc0_ps = ps_tp.tile([P, 2 * P], f32, tag="tp_ps")
                    nc.tensor.matmul(sc0_ps, k0, q01, start=True, stop=True)
                    sc0_r = scw.tile([P, 2 * P], bf16, tag="sc_r")
                    nc.vector.tensor_mul(out=sc0_r, in0=sc0_ps, in1=mask2)
                    sc1_ps = ps_tp.tile([P, P], f32, tag="tp_ps")
                    nc.tensor.matmul(sc1_ps, k1, q1, start=True, stop=True)
                    sc1_r = scw.tile([P, P], bf16, tag="sc1_r")
                    nc.vector.tensor_mul(out=sc1_r, in0=sc1_ps, in1=mask_t)
                    y_ps = ps_mm.tile([P, 2, HD], f32, tag="pst")
                    nc.tensor.matmul(y_ps[:, 0, :], sc0_r[:, 0:P], v0, start=True, stop=False)
                    nc.tensor.matmul(y_ps[:, 0, :], q0, Mt_s[pb:pb + HD, fi, :],
                                     start=False, stop=True)
                    nc.tensor.matmul(y_ps[:, 1, :], sc0_r[:, P:2 * P], v0, start=True, stop=False)
                    nc.tensor.matmul(y_ps[:, 1, :], sc1_r, v1, start=False, stop=False)
                    nc.tensor.matmul(y_ps[:, 1, :], q1, Mt_s[pb:pb + HD, fi, :],
                                     start=False, stop=True)
                    nc.scalar.copy(out=y[:, c0:c0 + 2, h * HD:(h + 1) * HD], in_=y_ps)
                    mt_ps = ps_scan.tile([HD, HD], f32, tag="mt")
                    nc.tensor.matmul(mt_ps, kc_s[:, 0, :], v0, start=True, stop=False)
                    nc.tensor.matmul(mt_ps, kc_s[:, 1, :], v1, start=False, stop=True)
                    nc.vector.tensor_add(out=Mt_f[pb:pb + HD, fi, :],
                                         in0=Mt_f[pb:pb + HD, fi, :], in1=mt_ps)
                    nc.vector.tensor_copy(out=Mt_s[pb:pb + HD, fi, :],
                                          in_=Mt_f[pb:pb + HD, fi, :])
            # epilogue: yT, attn-out + residual, LN2, h2T into persistent hT
            for m in range(ntt):
                yT_m = yTp.tile([P, D_T, P], bf16, tag="yT_m")
                for fidx in range(D_T):
                    tps = ps_tp.tile([P, P], bf16, tag="tp_ps")
                    nc.tensor.transpose(tps, y[:, m, fidx * P:(fidx + 1) * P], ident)
                    nc.scalar.copy(out=yT_m[:, fidx, :], in_=tps)
                x_t = xp.tile([P, DIM], f32, tag=f"x{m % 2}")
                nc.sync.dma_start(out=x_t, in_=x_own.ap()[tok0 + m * P:tok0 + (m + 1) * P, :])
                ti = (tok0 // P) + m
                x2f = x2fp.tile([P, DIM], f32, tag="x2f")
                for nh in range(2):
                    pst = ps_mm.tile([P, 512], f32, tag="pst")
                    for k in range(D_T):
                        nc.tensor.matmul(pst, yT_m[:, k, :], wo_sb[:, k, nh * 512:(nh + 1) * 512],
                                         start=(k == 0), stop=(k == D_T - 1))
                    nc.vector.tensor_add(out=x2f[:, nh * 512:(nh + 1) * 512],
                                         in0=x_t[:, nh * 512:(nh + 1) * 512], in1=pst)
                x2b = x2fp.tile([P, DIM], bf16, tag="x2b")
                nc.scalar.copy(out=x2b, in_=x2f)
                nc.scalar.dma_start(out=x2_d[ti], in_=x2b)
                h2_r = hrp.tile([P, DIM], bf16, tag="h2r")
                _layernorm_tile(nc, ln_w, x2f, g2, b2, eps_t, h2_r)
                for fidx in range(D_T):
                    tps = ps_tp.tile([P, P], bf16, tag="tp_ps")
                    nc.tensor.transpose(tps, h2_r[:, fidx * P:(fidx + 1) * P], ident)
                    nc.scalar.copy(out=hT[:, fidx, ti * P:(ti + 1) * P], in_=tps)

        ab_stack.close()

        # final level's output in bf16 (fp8 PE transpose is illegal, so
        # phase D transposes this instead of hT)
        hfin_pool = top.enter_context(tc.tile_pool(name="hfin", bufs=1))
        h_fin = hfin_pool.tile([P, D_T, t_own], bf16)

        # ---------------- Phase C: CMS FFN, fully SBUF-resident ----------------
        with ExitStack() as ffn:
            big = ffn.enter_context(tc.tile_pool(name="ffn_big", bufs=1))
            upg = big.tile([P, HHT, t_own], f8)         # gelu acts, one hidden half
            out_acc = big.tile([P, D_T, t_own], bf16)   # half-0 partials + b2 (true scale)
            bp = ffn.enter_context(tc.tile_pool(name="ffn_b", bufs=2))
            w1s = ffn.enter_context(tc.tile_pool(name="w1s", bufs=2))
            w2s = ffn.enter_context(tc.tile_pool(name="w2s", bufs=2))
            hsc = ffn.enter_context(tc.tile_pool(name="hsc", bufs=3))
            ps_up = ffn.enter_context(tc.tile_pool(name="ps_up", bufs=4, space="PSUM"))
            ps_dn = ffn.enter_context(tc.tile_pool(name="ps_dn", bufs=4, space="PSUM"))
            for lvl in range(LEVELS):
                b1_t = bp.tile([P, H_T], f32, tag="b1")
                nc.sync.dma_start(out=b1_t, in_=cms_b1.ap()[lvl].rearrange("(m p) -> p m", p=P))
                b2_t = bp.tile([P, D_T], f32, tag="b2")
                nc.sync.dma_start(out=b2_t, in_=cms_b2.ap()[lvl].rearrange("(m p) -> p m", p=P))
                w1_all = cms_w1.ap()[lvl].rearrange("(kt p) d -> p kt d", p=P)
                w2_all = cms_w2.ap()[lvl].rearrange("(kt p) d -> p kt d", p=P)
                for half in range(2):
                    for mg in range(4):  # 512 hidden cols per w1 tile
                        w1_t = w1s.tile([P, D_T, 512], f8, tag="w1t")
                        c0 = half * (HID // 2) + mg * 512
                        nc.sync.dma_start(out=w1_t, in_=w1_all[:, :, c0:c0 + 512])
                        for ml in range(4):
                            mh = mg * 4 + ml           # hidden tile within half
                            m_gl = half * HHT + mh     # global hidden tile
                            psl = [ps_up.tile([P, TT], f32, tag="up",
                                              name=f"up_{lvl}_{half}_{mh}_{i}")
                                   for i in range(n_tt)]
                            for kk in range(D_T // 2):
                                for tt in range(n_tt):
                                    nc.tensor.matmul(
                                        psl[tt], w1_t[:, 2 * kk:2 * kk + 2, ml * P:(ml + 1) * P],
                                        hT[:, 2 * kk:2 * kk + 2, tt * TT:(tt + 1) * TT],
                                        start=(kk == 0), stop=(kk == D_T // 2 - 1),
                                        perf_mode=PM.DoubleRow)
                            for tt in range(n_tt):
                                nc.scalar.activation(
                                    out=upg[:, mh, tt * TT:(tt + 1) * TT], in_=psl[tt],
                                    func=AF.Gelu_apprx_tanh,
                                    bias=b1_t[:, m_gl:m_gl + 1], scale=INV_W8)
                    for mdg in range(4):  # 256 output cols per w2 tile
                        w2_t = w2s.tile([P, HHT, 256], f8, tag="w2t")
                        nc.sync.dma_start(
                            out=w2_t,
                            in_=w2_all[:, half * HHT:(half + 1) * HHT, mdg * 256:(mdg + 1) * 256])
                        for mdl in range(2):
                            md = mdg * 2 + mdl
                            psl = [ps_dn.tile([P, TT], f32, tag="dn",
                                              name=f"dn_{lvl}_{half}_{md}_{i}")
                                   for i in range(n_tt)]
                            for kk in range(HHT // 2):
                                for tt in range(n_tt):
                                    nc.tensor.matmul(
                                        psl[tt], w2_t[:, 2 * kk:2 * kk + 2, mdl * P:(mdl + 1) * P],
                                        upg[:, 2 * kk:2 * kk + 2, tt * TT:(tt + 1) * TT],
                                        start=(kk == 0), stop=(kk == HHT // 2 - 1),
                                        perf_mode=PM.DoubleRow)
                            for tt in range(n_tt):
                                if half == 0:
                                    nc.scalar.activation(
                                        out=out_acc[:, md, tt * TT:(tt + 1) * TT], in_=psl[tt],
                                        func=AF.Identity, bias=b2_t[:, md:md + 1], scale=INV_W8)
                                else:
                                    htmp = hsc.tile([P, TT], bf16, tag="htmp")
                                    nc.scalar.activation(out=htmp, in_=psl[tt], func=AF.Identity,
                                                         bias=0.0, scale=INV_W8)
                                    dst = h_fin if lvl == LEVELS - 1 else hT
                                    nc.vector.tensor_add(
                                        out=dst[:, md, tt * TT:(tt + 1) * TT],
                                        in0=htmp, in1=out_acc[:, md, tt * TT:(tt + 1) * TT])

        # ---------------- Phase D: out = x2 + h^T ----------------
        with ExitStack() as ph:
            sb = ph.enter_context(tc.tile_pool(name="D_sb", bufs=3))
            ps = ph.enter_context(tc.tile_pool(name="D_ps", bufs=3, space="PSUM"))
            for t in range(n_own_t):
                x2_t = sb.tile([P, DIM], bf16, tag="D_x2")
                nc.sync.dma_start(out=x2_t, in_=x2_d[t])
                o_t = sb.tile([P, DIM], f32, tag="D_o")
                for fidx in range(D_T):
                    tps = ps.tile([P, P], bf16, tag="D_tp")
                    nc.tensor.transpose(tps, h_fin[:, fidx, t * P:(t + 1) * P], ident)
                    nc.vector.tensor_add(out=o_t[:, fidx * P:(fidx + 1) * P],
                                         in0=x2_t[:, fidx * P:(fidx + 1) * P],
                                         in1=tps)
                nc.scalar.dma_start(out=out.ap()[t * P:(t + 1) * P, :], in_=o_t)

    _split_multi_waits(nc)
    return nc


_NC_CACHE = {}
LAST_RESULT = None


def _get_nc(key, **kw):
    if key not in _NC_CACHE:
        _NC_CACHE[key] = build_kernel(**kw)
    return _NC_CACHE[key]


def kernel(x, ln1_g, ln1_b, wq, wk, wv, wo, ln2_g, ln2_b,
           cms_w1, cms_b1, cms_w2, cms_b2, **extra):
    import ml_dtypes
    bf = ml_dtypes.bfloat16
    f8h = ml_dtypes.float8_e4m3
    x = np.asarray(x, np.float32)
    maskT = np.triu(np.ones((CHUNK, CHUNK), np.float32))  # maskT[e,c] = e<=c
    common = {
        "wq": np.asarray(wq, bf), "wk": np.asarray(wk, bf),
        "wv": np.asarray(wv, bf), "wo": np.asarray(wo, bf),
        "ln1_g": np.asarray(ln1_g, np.float32), "ln1_b": np.asarray(ln1_b, np.float32),
        "ln2_g": np.asarray(ln2_g, np.float32), "ln2_b": np.asarray(ln2_b, np.float32),
        "cms_w1": (np.asarray(cms_w1, np.float32) * W8_SCALE).astype(f8h),
        "cms_b1": np.asarray(cms_b1, np.float32),
        "cms_w2": (np.asarray(cms_w2, np.float32) * W8_SCALE).astype(f8h),
        "cms_b2": np.asarray(cms_b2, np.float32),
        "maskT": maskT,
    }
    zeros_pre = np.zeros((T_PRE, DIM), np.float32)
    in_maps = []
    for c in range(N_CORES):
        b, half = c // 2, c % 2
        own = x[b, half * T_OWN:(half + 1) * T_OWN]
        pre = x[b, 0:T_PRE] if half else zeros_pre
        in_maps.append({**common, "x_own": np.ascontiguousarray(own),
                        "x_pre": np.ascontiguousarray(pre)})
    nc = _get_nc("full")
    res = run_bass_kernel_spmd(nc, in_maps, core_ids=list(range(N_CORES)))
    global LAST_RESULT
    LAST_RESULT = res
    out = np.empty((B, S, DIM), np.float32)
    for c in range(N_CORES):
        b, half = c // 2, c % 2
        out[b, half * T_OWN:(half + 1) * T_OWN] = res.results[c]["out"]
    return out



# revision 13
# speedup vs baseline: 1.3313x; 1.3313x over previous
"""HOPE block kernel for 8 Trainium2 NeuronCores — v3.

Sharding: 8 shards = (batch b in 0..3, sequence half in 0..1), 2048 tokens each.
v3 replaces v2's "rebuild prefix memory locally" phase A with a pairwise
AllGather of the linear-attention memory state M (256KB): core 2b runs the
first half of batch b's sequence and its post-scan M state IS the mid-sequence
memory the partner core 2b+1 needs.  Every core runs its local scan with M0=0;
after the gather, y += q @ M_remote is added (M_remote is scaled by a per-core
0/1 input so even cores add zero; the SPMD program stays uniform).

Other changes vs v2:
 - All weights are re-laid-out on the host so every DMA line is >=2KB
   contiguous (v2's 512B-1KB gather lines made the HBM counter run at ~95%
   during attention).  wq/wk/wv/wo stay SBUF-resident.
 - The scan is feature-major: y^T accumulates straight into the [feat, tok]
   layout the wo projection consumes (no y transposes), and each matmul
   processes a head PAIR via partition-offset tile packing (K=64 / M=64
   matmuls land on disjoint 64-row/64-col PE tile groups and run
   concurrently).
 - The CMS FFN down-projection accumulates over the full 4096 hidden dim in
   one PSUM group (no half-split partial buffer), and the last level emits
   token-major output directly so the final residual add + store needs no
   transposes.
"""
import sys
if '/opt/trn_rl_repo' not in sys.path:
    sys.path.insert(0, '/opt/trn_rl_repo')

from contextlib import ExitStack
import numpy as np


def _ensure_ntff_hook():
    """Register the axon NTFF profile hook when the image's antenv stub
    lacks `axon_hooks` — otherwise BASS_TRACE runs skip tracing and
    exec_time_ns comes back None. No-op when the real module exists."""
    import types, contextlib, ctypes, os
    try:
        from antenv.axon_hooks import get_axon_ntff_profile_hook  # noqa: F401
        return
    except ImportError:
        pass
    hook = None
    so_path = '/opt/axon/libaxon_pjrt.so'
    if os.path.exists(so_path):
        try:
            lib = ctypes.CDLL(so_path)
        except OSError:
            lib = None
        if lib is not None and hasattr(lib, 'axon_start_nrt_profile'):
            lib.axon_start_nrt_profile.argtypes = [
                ctypes.POINTER(ctypes.c_int64), ctypes.c_size_t]
            lib.axon_start_nrt_profile.restype = ctypes.c_int64
            lib.axon_stop_nrt_profile.argtypes = [ctypes.c_char_p]
            lib.axon_stop_nrt_profile.restype = ctypes.c_int64

            @contextlib.contextmanager
            def hook(output_dir, device_ids):
                import jax
                jax.devices()  # force PJRT init so the .so's client exists
                if device_ids:
                    ids = (ctypes.c_int64 * len(device_ids))(*device_ids)
                    rc = lib.axon_start_nrt_profile(ids, len(device_ids))
                else:
                    rc = lib.axon_start_nrt_profile(None, 0)
                if rc != 0:
                    raise RuntimeError(f"axon_start_nrt_profile rc={rc}")
                try:
                    yield
                finally:
                    n = lib.axon_stop_nrt_profile(str(output_dir).encode())
                    print(f"profile: {n} file(s) written to {output_dir}",
                          file=sys.stderr)

    try:
        import antenv
    except ImportError:
        return
    mod = types.ModuleType('antenv.axon_hooks')
    _h = hook
    mod.get_axon_ntff_profile_hook = lambda: _h
    mod.set_axon_ntff_profile_hook = lambda h: None
    sys.modules['antenv.axon_hooks'] = mod
    antenv.axon_hooks = mod


_ensure_ntff_hook()

import concourse.bass as bass
import concourse.tile as tile
from concourse import mybir
from concourse.bass_utils import run_bass_kernel_spmd
from concourse.masks import make_identity

f32 = mybir.dt.float32
bf16 = mybir.dt.bfloat16
f8 = mybir.dt.float8e4
AF = mybir.ActivationFunctionType
ALU = mybir.AluOpType
PM = mybir.MatmulPerfMode

W8_SCALE = 64.0     # host-side multiplier on fp8 FFN weights (clears the
INV_W8 = 1.0 / W8_SCALE  # e4m3 subnormal zone); un-done via activation scale

DIM = 1024
HEADS = 16
HD = 64
B, S = 4, 4096
LEVELS = 3
HID = 4 * DIM
CHUNK = 128
EPS = 1e-5
P = 128

N_CORES = 8
T_OWN = S // 2      # tokens per core
BLK = 512           # token block for the attention phase
D_T = DIM // P      # 8 feature tiles
H_T = HID // P      # 32 hidden tiles
N_BLK = T_OWN // BLK
NTT = BLK // P      # 4 token tiles per block
N_OWN_T = T_OWN // P
TT = 512            # FFN token tile

MAX_WAITS = 1

REPLICA_PAIRS = [[0, 1], [2, 3], [4, 5], [6, 7]]


def _split_multi_waits(nc, max_waits=MAX_WAITS):
    """Walrus in this toolchain encodes at most `max_waits` sem waits per
    instruction; split extra waits onto same-engine NOPs placed just before."""
    for f in nc.m.functions:
        for bb in f.blocks:
            insts = list(bb.instructions)
            if not any(
                i.sync_info and i.sync_info.on_wait and len(i.sync_info.on_wait) > max_waits
                for i in insts
            ):
                continue
            new = []
            for inst in insts:
                si = inst.sync_info
                waits = list(si.on_wait) if si and si.on_wait else []
                if len(waits) > max_waits:
                    head, rest = waits[:-max_waits], waits[-max_waits:]
                    while head:
                        chunk, head = head[:max_waits], head[max_waits:]
                        nop = mybir.InstNoOp(name=nc.get_next_instruction_name(), ins=[], outs=[])
                        nop.engine = inst.engine
                        nop.sync_info = mybir.SyncInfo(on_wait=chunk, on_update=[])
                        nc.register_instruction(nop, overwrite=True)
                        new.append(nop)
                    inst.sync_info = mybir.SyncInfo(
                        on_wait=rest, on_update=list(si.on_update) if si.on_update else [])
                new.append(inst)
            bb.instructions = new


def _layernorm_tile(nc, w, x_t, g_bc, b_bc, eps_t, out_r, affine=True):
    """LayerNorm of one [128, DIM] fp32 tile -> bf16 tile (token-major)."""
    BNF = nc.vector.BN_STATS_FMAX
    nsub = DIM // BNF
    stats = w.tile([P, nsub, nc.vector.BN_STATS_DIM], f32, tag="ln_stats")
    xg = x_t[:].rearrange("p (s f) -> p s f", f=BNF)
    for s_ in range(nsub):
        nc.vector.bn_stats(out=stats[:, s_, :], in_=xg[:, s_, :])
    mv = w.tile([P, nc.vector.BN_AGGR_DIM], f32, tag="ln_mv")
    nc.vector.bn_aggr(out=mv, in_=stats)
    rstd = w.tile([P, 1], f32, tag="ln_rstd")
    nc.scalar.activation(out=rstd, in_=mv[:, 1:2], func=AF.Sqrt, bias=eps_t, scale=1.0)
    nc.vector.reciprocal(out=rstd, in_=rstd)
    if affine:
        tmp = w.tile([P, DIM], f32, tag="ln_tmp")
        nc.vector.tensor_scalar(out=tmp, in0=x_t, scalar1=mv[:, 0:1], scalar2=rstd,
                                op0=ALU.subtract, op1=ALU.mult)
        nc.vector.tensor_mul(out=tmp, in0=tmp, in1=g_bc)
        nc.vector.tensor_add(out=out_r, in0=tmp, in1=b_bc)
    else:
        nc.vector.tensor_scalar(out=out_r, in0=x_t, scalar1=mv[:, 0:1], scalar2=rstd,
                                op0=ALU.subtract, op1=ALU.mult)


def build_kernel(ln1_affine=True, ln2_affine=True, b2fin_zero=False, pairs=True):
    nc = bass.Bass(num_devices=N_CORES)
    groups = REPLICA_PAIRS if pairs else [[c] for c in range(N_CORES)]
    gsz = 2 if pairs else 1

    x_own = nc.dram_tensor("x_own", [T_OWN, DIM], f32, kind="ExternalInput")
    m_scale = nc.dram_tensor("m_scale", [1], f32, kind="ExternalInput")
    # [128, kt, d]: row (kt*128+p) of the original [DIM, DIM] weight
    wq_l = nc.dram_tensor("wq_l", [P, D_T, DIM], bf16, kind="ExternalInput")
    wk_l = nc.dram_tensor("wk_l", [P, D_T, DIM], bf16, kind="ExternalInput")
    wv_l = nc.dram_tensor("wv_l", [P, D_T, DIM], bf16, kind="ExternalInput")
    wo_l = nc.dram_tensor("wo_l", [P, D_T, DIM], bf16, kind="ExternalInput")
    ln1_g = nc.dram_tensor("ln1_g", [DIM], f32, kind="ExternalInput")
    ln1_b = nc.dram_tensor("ln1_b", [DIM], f32, kind="ExternalInput")
    ln2_g = nc.dram_tensor("ln2_g", [DIM], f32, kind="ExternalInput")
    ln2_b = nc.dram_tensor("ln2_b", [DIM], f32, kind="ExternalInput")
    # [lvl, 128, half, kt, 2048]: w1[lvl, kt*128+p, half*2048+c]
    w1_l = nc.dram_tensor("w1_l", [LEVELS, P, 2, D_T, HID // 2], f8, kind="ExternalInput")
    # [lvl, 128, half, kt16, 1024]: w2[lvl, half*2048+kt*128+p, d]
    w2_l = nc.dram_tensor("w2_l", [LEVELS, P, 2, H_T // 2, DIM], f8, kind="ExternalInput")
    b1_l = nc.dram_tensor("b1_l", [LEVELS, P, H_T], f32, kind="ExternalInput")
    b2_l = nc.dram_tensor("b2_l", [LEVELS - 1, P, D_T], f32, kind="ExternalInput")
    b2fin = nc.dram_tensor("b2fin", [DIM], f32, kind="ExternalInput")
    maskT = nc.dram_tensor("maskT", [CHUNK, CHUNK], f32, kind="ExternalInput")
    out = nc.dram_tensor("out", [T_OWN, DIM], f32, kind="ExternalOutput")

    with tile.TileContext(nc) as tc, ExitStack() as top:
        consts = top.enter_context(tc.tile_pool(name="consts", bufs=1))
        ident_f = consts.tile([P, P], f32)
        make_identity(nc, ident_f)
        ident = consts.tile([P, P], bf16)
        nc.vector.tensor_copy(out=ident, in_=ident_f)
        eps_t = consts.tile([P, 1], f32)
        nc.vector.memset(eps_t, EPS)
        # combined super-chunk score mask [e, S0(c0)|S0(c1)|S1]:
        # causal for (k0,q0), all-1 for (k0,q1), causal for (k1,q1)
        maskC = consts.tile([CHUNK, 3 * CHUNK], f32)
        nc.sync.dma_start(out=maskC[:, 0:CHUNK], in_=maskT.ap())
        nc.vector.memset(maskC[:, CHUNK:2 * CHUNK], 1.0)
        nc.sync.dma_start(out=maskC[:, 2 * CHUNK:3 * CHUNK], in_=maskT.ap())
        g1 = b1 = g2 = b2 = None
        if ln1_affine:
            g1 = consts.tile([P, DIM], bf16)
            b1 = consts.tile([P, DIM], bf16)
            nc.sync.dma_start(out=g1, in_=ln1_g.ap()[None, :].partition_broadcast(P).opt())
            nc.sync.dma_start(out=b1, in_=ln1_b.ap()[None, :].partition_broadcast(P).opt())
        if ln2_affine:
            g2 = consts.tile([P, DIM], bf16)
            b2 = consts.tile([P, DIM], bf16)
            nc.sync.dma_start(out=g2, in_=ln2_g.ap()[None, :].partition_broadcast(P).opt())
            nc.sync.dma_start(out=b2, in_=ln2_b.ap()[None, :].partition_broadcast(P).opt())
        msc = consts.tile([P, 1], f32)
        nc.sync.dma_start(out=msc, in_=m_scale.ap()[None, :].partition_broadcast(P).opt())

        # persistent across phases: FFN input h2^T (feature-major, fp8)
        persist = top.enter_context(tc.tile_pool(name="persist", bufs=1))
        hT = persist.tile([P, D_T, T_OWN], f8)
        dram = top.enter_context(tc.tile_pool(name="dram", bufs=1, space="DRAM"))
        x2_d = dram.tile([N_OWN_T, P, DIM], bf16)
        mt_d = dram.tile([P, D_T * HD], f32)
        ag_d = dram.tile([gsz, P, D_T * HD], f32)

        # qT / yT persist until the post-collective fixup + wo projection
        mid = ExitStack()
        qyp = mid.enter_context(tc.tile_pool(name="qyp", bufs=1))
        qT_p = qyp.tile([P, D_T, T_OWN], bf16)
        yT_p = qyp.tile([P, D_T, T_OWN], bf16)
        wo_pool = mid.enter_context(tc.tile_pool(name="wo_pool", bufs=1))
        wo_s = wo_pool.tile([P, D_T, DIM], bf16)
        nc.sync.dma_start(out=wo_s, in_=wo_l.ap())

        # ---------------- attention ----------------
        ab = ExitStack()
        wqkv = ab.enter_context(tc.tile_pool(name="wqkv", bufs=1))
        wq_s = wqkv.tile([P, D_T, DIM], bf16)
        wk_s = wqkv.tile([P, D_T, DIM], bf16)
        wv_s = wqkv.tile([P, D_T, DIM], bf16)
        nc.sync.dma_start(out=wq_s, in_=wq_l.ap())
        nc.sync.dma_start(out=wk_s, in_=wk_l.ap())
        nc.sync.dma_start(out=wv_s, in_=wv_l.ap())

        mt_pool = ab.enter_context(tc.tile_pool(name="mt", bufs=1))
        Mt_f = mt_pool.tile([P, D_T, HD], f32)   # partitions = d of head pair
        Mt_s = mt_pool.tile([P, D_T, HD], bf16)
        nc.vector.memset(Mt_f, 0.0)
        nc.vector.memset(Mt_s, 0.0)

        ln_w = ab.enter_context(tc.tile_pool(name="ln_w", bufs=2))
        xp = ab.enter_context(tc.tile_pool(name="xp", bufs=2))
        hrp = ab.enter_context(tc.tile_pool(name="hrp", bufs=1))
        h1Tp = ab.enter_context(tc.tile_pool(name="h1Tp", bufs=2))
        kvp = ab.enter_context(tc.tile_pool(name="kvp", bufs=2))
        vp = ab.enter_context(tc.tile_pool(name="vp", bufs=1))
        scw = ab.enter_context(tc.tile_pool(name="scw", bufs=2))
        # PSUM budget (8 banks): tp 2 + mm 2 + S 2 + ymt 1 + kc 1
        ps_tp = ab.enter_context(tc.tile_pool(name="ps_tp", bufs=2, space="PSUM"))
        ps_mm = ab.enter_context(tc.tile_pool(name="ps_mm", bufs=2, space="PSUM"))
        ps_S = ab.enter_context(tc.tile_pool(name="ps_S", bufs=1, space="PSUM"))
        ps_ymt = ab.enter_context(tc.tile_pool(name="ps_ymt", bufs=1, space="PSUM"))
        ps_kc = ab.enter_context(tc.tile_pool(name="ps_kc", bufs=1, space="PSUM"))

        for blk in range(N_BLK):
            tok0 = blk * BLK
            # LayerNorm1 + transpose into feature-major h1T
            h1T = h1Tp.tile([P, D_T, BLK], bf16, tag="h1T")
            for t in range(NTT):
                x_t = xp.tile([P, DIM], f32, tag="x")
                nc.sync.dma_start(out=x_t, in_=x_own.ap()[tok0 + t * P:tok0 + (t + 1) * P, :])
                h_r = hrp.tile([P, DIM], bf16, tag="h1r")
                _layernorm_tile(nc, ln_w, x_t, g1, b1, eps_t, h_r, affine=ln1_affine)
                for fp in range(D_T // 2):
                    tps = ps_tp.tile([P, 2, P], bf16, tag="tp")
                    for j in range(2):
                        fi = 2 * fp + j
                        nc.tensor.transpose(tps[:, j, :], h_r[:, fi * P:(fi + 1) * P], ident)
                    nc.vector.tensor_copy(out=h1T[:, 2 * fp:2 * fp + 2, t * P:(t + 1) * P],
                                          in_=tps)
            # q, k: feature-major [feat, tok]
            kT = kvp.tile([P, D_T, BLK], bf16, tag="kT")
            for (w_s, dst, off) in ((wq_s, qT_p, tok0), (wk_s, kT, 0)):
                for m in range(D_T):
                    pst = ps_mm.tile([P, BLK], f32, tag="pst")
                    for k in range(D_T):
                        nc.tensor.matmul(pst, w_s[:, k, m * P:(m + 1) * P], h1T[:, k, :],
                                         start=(k == 0), stop=(k == D_T - 1))
                    nc.scalar.copy(out=dst[:, m, off:off + BLK], in_=pst)
            # v: token-major [tok, feat]
            v = vp.tile([P, NTT, DIM], bf16, tag="v")
            for ti in range(NTT):
                for nh in range(2):
                    pst = ps_mm.tile([P, 512], f32, tag="pst")
                    for k in range(D_T):
                        nc.tensor.matmul(pst, h1T[:, k, ti * P:(ti + 1) * P],
                                         wv_s[:, k, nh * 512:(nh + 1) * 512],
                                         start=(k == 0), stop=(k == D_T - 1))
                    nc.scalar.copy(out=v[:, ti, nh * 512:(nh + 1) * 512], in_=pst)
            # scan: super-chunks of 256 tokens, head-pair packed, feature-major y
            for sch in range(NTT // 2):
                s0 = sch * 2 * P          # in-block offset of chunk c0
                q0 = tok0 + s0            # global offset
                for fi in range(D_T):
                    h0c = (2 * fi) * HD   # head col offsets in token-major v
                    h1c = (2 * fi + 1) * HD
                    # k chunk transposes (both heads at once: [tok, d-pair])
                    kc_ps = ps_kc.tile([P, 2, P], bf16, tag="kc")
                    nc.tensor.transpose(kc_ps[:, 0, :], kT[:, fi, s0:s0 + P], ident)
                    nc.tensor.transpose(kc_ps[:, 1, :], kT[:, fi, s0 + P:s0 + 2 * P], ident)
                    kc = scw.tile([P, 2, P], bf16, tag="kc")
                    nc.scalar.copy(out=kc, in_=kc_ps)
                    # scores [head, S0(256)|S1(128)|pad] (K=64 -> row-group packed
                    # pair; padded to 512 so each head's region is bank-aligned)
                    s_ps = ps_S.tile([P, 2, 4 * P], f32, tag="s")
                    for hh in range(2):
                        pb = hh * HD
                        nc.tensor.matmul(s_ps[:, hh, 0:2 * P],
                                         kT[pb:pb + HD, fi, s0:s0 + P],
                                         qT_p[pb:pb + HD, fi, q0:q0 + 2 * P],
                                         start=True, stop=True)
                        nc.tensor.matmul(s_ps[:, hh, 2 * P:3 * P],
                                         kT[pb:pb + HD, fi, s0 + P:s0 + 2 * P],
                                         qT_p[pb:pb + HD, fi, q0 + P:q0 + 2 * P],
                                         start=True, stop=True)
                    s_r = scw.tile([P, 2, 3 * P], bf16, tag="s_r")
                    nc.vector.tensor_mul(out=s_r[:, 0, :], in0=s_ps[:, 0, 0:3 * P], in1=maskC)
                    nc.vector.tensor_mul(out=s_r[:, 1, :], in0=s_ps[:, 1, 0:3 * P], in1=maskC)
                    # y^T accumulation (intra col-packed + memory diag-packed)
                    # and M update (col-packed), sharing one PSUM bank
                    ymt = ps_ymt.tile([P, 2 * P + HD], f32, tag="ymt")
                    y_ps = ymt[:, 0:2 * P]
                    mt_ps = ymt[:, 2 * P:2 * P + HD]
                    nc.tensor.matmul(y_ps[0:HD, :], v[:, sch * 2, h0c:h0c + HD],
                                     s_r[:, 0, 0:2 * P], start=True, stop=False)
                    nc.tensor.matmul(y_ps[HD:P, :], v[:, sch * 2, h1c:h1c + HD],
                                     s_r[:, 1, 0:2 * P], start=True, stop=False)
                    nc.tensor.matmul(y_ps[0:HD, P:2 * P], v[:, sch * 2 + 1, h0c:h0c + HD],
                                     s_r[:, 0, 2 * P:3 * P], start=False, stop=False)
                    nc.tensor.matmul(y_ps[HD:P, P:2 * P], v[:, sch * 2 + 1, h1c:h1c + HD],
                                     s_r[:, 1, 2 * P:3 * P], start=False, stop=False)
                    nc.tensor.matmul(y_ps[0:HD, :], Mt_s[0:HD, fi, :],
                                     qT_p[0:HD, fi, q0:q0 + 2 * P], start=False, stop=True)
                    nc.tensor.matmul(y_ps[HD:P, :], Mt_s[HD:P, fi, :],
                                     qT_p[HD:P, fi, q0:q0 + 2 * P], start=False, stop=True)
                    nc.scalar.copy(out=yT_p[:, fi, q0:q0 + 2 * P], in_=y_ps)
                    # M update (col-packed pair per chunk)
                    nc.tensor.matmul(mt_ps[0:HD, :], kc[:, 0, 0:HD],
                                     v[:, sch * 2, h0c:h0c + HD], start=True, stop=False)
                    nc.tensor.matmul(mt_ps[HD:P, :], kc[:, 0, HD:P],
                                     v[:, sch * 2, h1c:h1c + HD], start=True, stop=False)
                    nc.tensor.matmul(mt_ps[0:HD, :], kc[:, 1, 0:HD],
                                     v[:, sch * 2 + 1, h0c:h0c + HD], start=False, stop=True)
                    nc.tensor.matmul(mt_ps[HD:P, :], kc[:, 1, HD:P],
                                     v[:, sch * 2 + 1, h1c:h1c + HD], start=False, stop=True)
                    nc.vector.tensor_add(out=Mt_f[:, fi, :], in0=Mt_f[:, fi, :], in1=mt_ps)
                    nc.vector.tensor_copy(out=Mt_s[:, fi, :], in_=Mt_f[:, fi, :])

        # ---------------- pairwise memory-state exchange ----------------
        nc.gpsimd.dma_start(mt_d[:], Mt_f[:].rearrange("p a b -> p (a b)"))
        nc.gpsimd.collective_compute(
            "AllGather",
            mybir.AluOpType.bypass,
            replica_groups=groups,
            ins=[mt_d.opt()],
            outs=[ag_d.opt()],
        )
        mrem_f = mt_pool.tile([P, D_T, HD], f32)
        nc.gpsimd.dma_start(mrem_f[:].rearrange("p a b -> p (a b)"), ag_d[0])
        # even cores start the sequence: scale their received M to zero
        nc.vector.tensor_scalar_mul(out=mrem_f, in0=mrem_f, scalar1=msc[:, 0:1])
        mrem = mt_pool.tile([P, D_T, HD], bf16)
        nc.vector.tensor_copy(out=mrem, in_=mrem_f)
        # y += q @ M_remote over all own tokens
        for fi in range(D_T):
            for g in range(T_OWN // 512):
                ps = ps_mm.tile([P, 512], f32, tag="pst")
                nc.tensor.matmul(ps[0:HD, :], mrem[0:HD, fi, :],
                                 qT_p[0:HD, fi, g * 512:(g + 1) * 512],
                                 start=True, stop=True)
                nc.tensor.matmul(ps[HD:P, :], mrem[HD:P, fi, :],
                                 qT_p[HD:P, fi, g * 512:(g + 1) * 512],
                                 start=True, stop=True)
                nc.vector.tensor_add(out=yT_p[:, fi, g * 512:(g + 1) * 512],
                                     in0=yT_p[:, fi, g * 512:(g + 1) * 512], in1=ps)

        ab.close()

        # ---------------- epilogue: wo, residual, LN2, h2^T ----------------
        ep = ExitStack()
        epw = ep.enter_context(tc.tile_pool(name="epw", bufs=2))
        ep1 = ep.enter_context(tc.tile_pool(name="ep1", bufs=1))
        ln_w2 = ep.enter_context(tc.tile_pool(name="ln_w2", bufs=2))
        ps_wo = ep.enter_context(tc.tile_pool(name="ps_wo", bufs=4, space="PSUM"))
        ps_t2 = ep.enter_context(tc.tile_pool(name="ps_t2", bufs=4, space="PSUM"))
        b2bc = None
        if not b2fin_zero:
            b2bc = ep1.tile([P, DIM], f32)
            nc.sync.dma_start(out=b2bc, in_=b2fin.ap()[None, :].partition_broadcast(P).opt())
        for ti in range(N_OWN_T):
            x_t = epw.tile([P, DIM], f32, tag="ex")
            nc.sync.dma_start(out=x_t, in_=x_own.ap()[ti * P:(ti + 1) * P, :])
            x2f = epw.tile([P, DIM], f32, tag="x2f")
            for nh in range(2):
                pst = ps_wo.tile([P, 512], f32, tag="wo")
                for k in range(D_T):
                    nc.tensor.matmul(pst, yT_p[:, k, ti * P:(ti + 1) * P],
                                     wo_s[:, k, nh * 512:(nh + 1) * 512],
                                     start=(k == 0), stop=(k == D_T - 1))
                nc.vector.tensor_add(out=x2f[:, nh * 512:(nh + 1) * 512],
                                     in0=x_t[:, nh * 512:(nh + 1) * 512], in1=pst)
            # spill x2 (+ final-level FFN bias, pre-added) for the output residual
            x2b = epw.tile([P, DIM], bf16, tag="x2b")
            if b2fin_zero:
                nc.scalar.copy(out=x2b, in_=x2f)
            else:
                nc.vector.tensor_add(out=x2b, in0=x2f, in1=b2bc)
            nc.scalar.dma_start(out=x2_d[ti], in_=x2b)
            h2_r = epw.tile([P, DIM], bf16, tag="h2r")
            _layernorm_tile(nc, ln_w2, x2f, g2, b2, eps_t, h2_r, affine=ln2_affine)
            for fi in range(D_T):
                tps = ps_t2.tile([P, P], bf16, tag="tp2")
                nc.tensor.transpose(tps, h2_r[:, fi * P:(fi + 1) * P], ident)
                nc.scalar.copy(out=hT[:, fi, ti * P:(ti + 1) * P], in_=tps)
        ep.close()
        mid.close()

        # ---------------- CMS FFN ----------------
        n_tt = T_OWN // TT
        with ExitStack() as ffn:
            w1s = ffn.enter_context(tc.tile_pool(name="w1s", bufs=1))
            w2s = ffn.enter_context(tc.tile_pool(name="w2s", bufs=1))
            bp = ffn.enter_context(tc.tile_pool(name="ffn_b", bufs=2))
            big = ffn.enter_context(tc.tile_pool(name="ffn_big", bufs=1))
            upg = big.tile([P, H_T, T_OWN], f8)     # gelu acts, full hidden
            ow = ffn.enter_context(tc.tile_pool(name="ow", bufs=3))
            ps_up = ffn.enter_context(tc.tile_pool(name="ps_up", bufs=4, space="PSUM"))
            ps_dn = ffn.enter_context(tc.tile_pool(name="ps_dn", bufs=4, space="PSUM"))
            for lvl in range(LEVELS):
                w1_t = w1s.tile([P, 2, D_T, HID // 2], f8, tag="w1t")
                for half in range(2):
                    nc.sync.dma_start(out=w1_t[:, half], in_=w1_l.ap()[lvl, :, half])
                w2_t = w2s.tile([P, H_T, DIM], f8, tag="w2t")
                for half in range(2):
                    nc.sync.dma_start(
                        out=w2_t[:, half * (H_T // 2):(half + 1) * (H_T // 2), :],
                        in_=w2_l.ap()[lvl, :, half])
                b1_t = bp.tile([P, H_T], f32, tag="b1")
                nc.sync.dma_start(out=b1_t, in_=b1_l.ap()[lvl])
                if lvl < LEVELS - 1:
                    b2_t = bp.tile([P, D_T], f32, tag="b2")
                    nc.sync.dma_start(out=b2_t, in_=b2_l.ap()[lvl])
                # up: h @ w1 -> gelu (feature-major hidden)
                for mh in range(H_T):
                    half, ml = mh // (H_T // 2), mh % (H_T // 2)
                    w1_lhs = w1_t[:, half]
                    for tt in range(n_tt):
                        psl = ps_up.tile([P, TT], f32, tag="up", name=f"up_{lvl}_{mh}_{tt}")
                        for kk in range(D_T // 2):
                            nc.tensor.matmul(
                                psl, w1_lhs[:, 2 * kk:2 * kk + 2, ml * P:(ml + 1) * P],
                                hT[:, 2 * kk:2 * kk + 2, tt * TT:(tt + 1) * TT],
                                start=(kk == 0), stop=(kk == D_T // 2 - 1),
                                perf_mode=PM.DoubleRow)
                        nc.scalar.activation(
                            out=upg[:, mh, tt * TT:(tt + 1) * TT], in_=psl,
                            func=AF.Gelu_apprx_tanh,
                            bias=b1_t[:, mh:mh + 1], scale=INV_W8)
                if lvl < LEVELS - 1:
                    # down, feature-major back into hT (full-hidden PSUM group)
                    for md in range(D_T):
                        for tt in range(n_tt):
                            psl = ps_dn.tile([P, TT], f32, tag="dn", name=f"dn_{lvl}_{md}_{tt}")
                            for kk in range(H_T // 2):
                                nc.tensor.matmul(
                                    psl, w2_t[:, 2 * kk:2 * kk + 2, md * P:(md + 1) * P],
                                    upg[:, 2 * kk:2 * kk + 2, tt * TT:(tt + 1) * TT],
                                    start=(kk == 0), stop=(kk == H_T // 2 - 1),
                                    perf_mode=PM.DoubleRow)
                            nc.scalar.activation(
                                out=hT[:, md, tt * TT:(tt + 1) * TT], in_=psl,
                                func=AF.Identity, bias=b2_t[:, md:md + 1], scale=INV_W8)
                else:
                    # last level: token-major output, fused residual + store
                    for ti in range(N_OWN_T):
                        x2_t = ow.tile([P, DIM], bf16, tag="ox2")
                        nc.sync.dma_start(out=x2_t, in_=x2_d[ti])
                        o_t = ow.tile([P, DIM], f32, tag="oo")
                        for nh in range(2):
                            psl = ps_dn.tile([P, 512], f32, tag="dn", name=f"fin_{ti}_{nh}")
                            for kk in range(H_T // 2):
                                nc.tensor.matmul(
                                    psl, upg[:, 2 * kk:2 * kk + 2, ti * P:(ti + 1) * P],
                                    w2_t[:, 2 * kk:2 * kk + 2, nh * 512:(nh + 1) * 512],
                                    start=(kk == 0), stop=(kk == H_T // 2 - 1),
                                    perf_mode=PM.DoubleRow)
                            htmp = ow.tile([P, 512], bf16, tag="oh")
                            nc.scalar.activation(out=htmp, in_=psl, func=AF.Identity,
                                                 bias=0.0, scale=INV_W8)
                            nc.vector.tensor_add(out=o_t[:, nh * 512:(nh + 1) * 512],
                                                 in0=x2_t[:, nh * 512:(nh + 1) * 512],
                                                 in1=htmp)
                        nc.scalar.dma_start(out=out.ap()[ti * P:(ti + 1) * P, :], in_=o_t)

    _split_multi_waits(nc)
    return nc


_NC_CACHE = {}
LAST_RESULT = None


def _get_nc(key, **kw):
    if key not in _NC_CACHE:
        _NC_CACHE[key] = build_kernel(**kw)
    return _NC_CACHE[key]


def kernel(x, ln1_g, ln1_b, wq, wk, wv, wo, ln2_g, ln2_b,
           cms_w1, cms_b1, cms_w2, cms_b2, **extra):
    import ml_dtypes
    bf = ml_dtypes.bfloat16
    f8h = ml_dtypes.float8_e4m3
    x = np.asarray(x, np.float32)
    maskT = np.triu(np.ones((CHUNK, CHUNK), np.float32))  # maskT[e,c] = e<=c

    def wlin(w):  # [DIM, DIM] -> [128, kt, DIM]
        return np.ascontiguousarray(
            np.asarray(w, np.float32).reshape(D_T, P, DIM).transpose(1, 0, 2).astype(bf))

    w1s = (np.asarray(cms_w1, np.float32) * W8_SCALE)
    w1_h = np.ascontiguousarray(
        w1s.reshape(LEVELS, D_T, P, 2, HID // 2).transpose(0, 2, 3, 1, 4).astype(f8h))
    w2s = (np.asarray(cms_w2, np.float32) * W8_SCALE)
    w2_h = np.ascontiguousarray(
        w2s.reshape(LEVELS, 2, H_T // 2, P, DIM).transpose(0, 3, 1, 2, 4).astype(f8h))
    b1_h = np.ascontiguousarray(
        np.asarray(cms_b1, np.float32).reshape(LEVELS, H_T, P).transpose(0, 2, 1))
    b2a = np.asarray(cms_b2, np.float32)
    b2_h = np.ascontiguousarray(b2a[:LEVELS - 1].reshape(LEVELS - 1, D_T, P).transpose(0, 2, 1))
    b2fin = np.ascontiguousarray(b2a[LEVELS - 1])

    ln1_g = np.asarray(ln1_g, np.float32)
    ln1_b = np.asarray(ln1_b, np.float32)
    ln2_g = np.asarray(ln2_g, np.float32)
    ln2_b = np.asarray(ln2_b, np.float32)
    ln1_affine = not (np.all(ln1_g == 1.0) and np.all(ln1_b == 0.0))
    ln2_affine = not (np.all(ln2_g == 1.0) and np.all(ln2_b == 0.0))
    b2fin_zero = bool(np.all(b2fin == 0.0))

    common = {
        "wq_l": wlin(wq), "wk_l": wlin(wk), "wv_l": wlin(wv), "wo_l": wlin(wo),
        "ln1_g": ln1_g, "ln1_b": ln1_b, "ln2_g": ln2_g, "ln2_b": ln2_b,
        "w1_l": w1_h, "w2_l": w2_h, "b1_l": b1_h, "b2_l": b2_h, "b2fin": b2fin,
        "maskT": maskT,
    }
    in_maps = []
    for c in range(N_CORES):
        b, half = c // 2, c % 2
        own = x[b, half * T_OWN:(half + 1) * T_OWN]
        in_maps.append({**common, "x_own": np.ascontiguousarray(own),
                        "m_scale": np.array([float(half)], np.float32)})
    nc = _get_nc(("v3", ln1_affine, ln2_affine, b2fin_zero),
                 ln1_affine=ln1_affine, ln2_affine=ln2_affine, b2fin_zero=b2fin_zero)
    res = run_bass_kernel_spmd(nc, in_maps, core_ids=list(range(N_CORES)))
    global LAST_RESULT
    LAST_RESULT = res
    out = np.empty((B, S, DIM), np.float32)
    for c in range(N_CORES):
        b, half = c // 2, c % 2
        out[b, half * T_OWN:(half + 1) * T_OWN] = res.results[c]["out"]
    return out


# revision 19
# speedup vs baseline: 1.3655x; 1.0257x over previous
"""HOPE block kernel for 8 Trainium2 NeuronCores — v3.

Sharding: 8 shards = (batch b in 0..3, sequence half in 0..1), 2048 tokens each.
v3 replaces v2's "rebuild prefix memory locally" phase A with a pairwise
AllGather of the linear-attention memory state M (256KB): core 2b runs the
first half of batch b's sequence and its post-scan M state IS the mid-sequence
memory the partner core 2b+1 needs.  Every core runs its local scan with M0=0;
after the gather, y += q @ M_remote is added (M_remote is scaled by a per-core
0/1 input so even cores add zero; the SPMD program stays uniform).

Other changes vs v2:
 - All weights are re-laid-out on the host so every DMA line is >=2KB
   contiguous (v2's 512B-1KB gather lines made the HBM counter run at ~95%
   during attention).  wq/wk/wv/wo stay SBUF-resident.
 - The scan is feature-major: y^T accumulates straight into the [feat, tok]
   layout the wo projection consumes (no y transposes), and each matmul
   processes a head PAIR via partition-offset tile packing (K=64 / M=64
   matmuls land on disjoint 64-row/64-col PE tile groups and run
   concurrently).
 - The CMS FFN down-projection accumulates over the full 4096 hidden dim in
   one PSUM group (no half-split partial buffer), and the last level emits
   token-major output directly so the final residual add + store needs no
   transposes.
"""
import sys
if '/opt/trn_rl_repo' not in sys.path:
    sys.path.insert(0, '/opt/trn_rl_repo')

from contextlib import ExitStack
import numpy as np


def _ensure_ntff_hook():
    """Register the axon NTFF profile hook when the image's antenv stub
    lacks `axon_hooks` — otherwise BASS_TRACE runs skip tracing and
    exec_time_ns comes back None. No-op when the real module exists."""
    import types, contextlib, ctypes, os
    try:
        from antenv.axon_hooks import get_axon_ntff_profile_hook  # noqa: F401
        return
    except ImportError:
        pass
    hook = None
    so_path = '/opt/axon/libaxon_pjrt.so'
    if os.path.exists(so_path):
        try:
            lib = ctypes.CDLL(so_path)
        except OSError:
            lib = None
        if lib is not None and hasattr(lib, 'axon_start_nrt_profile'):
            lib.axon_start_nrt_profile.argtypes = [
                ctypes.POINTER(ctypes.c_int64), ctypes.c_size_t]
            lib.axon_start_nrt_profile.restype = ctypes.c_int64
            lib.axon_stop_nrt_profile.argtypes = [ctypes.c_char_p]
            lib.axon_stop_nrt_profile.restype = ctypes.c_int64

            @contextlib.contextmanager
            def hook(output_dir, device_ids):
                import jax
                jax.devices()  # force PJRT init so the .so's client exists
                if device_ids:
                    ids = (ctypes.c_int64 * len(device_ids))(*device_ids)
                    rc = lib.axon_start_nrt_profile(ids, len(device_ids))
                else:
                    rc = lib.axon_start_nrt_profile(None, 0)
                if rc != 0:
                    raise RuntimeError(f"axon_start_nrt_profile rc={rc}")
                try:
                    yield
                finally:
                    n = lib.axon_stop_nrt_profile(str(output_dir).encode())
                    print(f"profile: {n} file(s) written to {output_dir}",
                          file=sys.stderr)

    try:
        import antenv
    except ImportError:
        return
    mod = types.ModuleType('antenv.axon_hooks')
    _h = hook
    mod.get_axon_ntff_profile_hook = lambda: _h
    mod.set_axon_ntff_profile_hook = lambda h: None
    sys.modules['antenv.axon_hooks'] = mod
    antenv.axon_hooks = mod


_ensure_ntff_hook()

import concourse.bass as bass
import concourse.tile as tile
from concourse import mybir
from concourse.bass_utils import run_bass_kernel_spmd
from concourse.masks import make_identity

f32 = mybir.dt.float32
bf16 = mybir.dt.bfloat16
f8 = mybir.dt.float8e4
AF = mybir.ActivationFunctionType
ALU = mybir.AluOpType
PM = mybir.MatmulPerfMode

W8_SCALE = 64.0     # host-side multiplier on fp8 FFN weights (clears the
INV_W8 = 1.0 / W8_SCALE  # e4m3 subnormal zone); un-done via activation scale

DIM = 1024
HEADS = 16
HD = 64
B, S = 4, 4096
LEVELS = 3
HID = 4 * DIM
CHUNK = 128
EPS = 1e-5
P = 128

N_CORES = 8
T_OWN = S // 2      # tokens per core
BLK = 512           # token block for the attention phase
D_T = DIM // P      # 8 feature tiles
H_T = HID // P      # 32 hidden tiles
N_BLK = T_OWN // BLK
NTT = BLK // P      # 4 token tiles per block
N_OWN_T = T_OWN // P
TT = 512            # FFN token tile

MAX_WAITS = 1

REPLICA_PAIRS = [[0, 1], [2, 3], [4, 5], [6, 7]]


def _split_multi_waits(nc, max_waits=MAX_WAITS):
    """Walrus in this toolchain encodes at most `max_waits` sem waits per
    instruction; split extra waits onto same-engine NOPs placed just before."""
    for f in nc.m.functions:
        for bb in f.blocks:
            insts = list(bb.instructions)
            if not any(
                i.sync_info and i.sync_info.on_wait and len(i.sync_info.on_wait) > max_waits
                for i in insts
            ):
                continue
            new = []
            for inst in insts:
                si = inst.sync_info
                waits = list(si.on_wait) if si and si.on_wait else []
                if len(waits) > max_waits:
                    head, rest = waits[:-max_waits], waits[-max_waits:]
                    while head:
                        chunk, head = head[:max_waits], head[max_waits:]
                        nop = mybir.InstNoOp(name=nc.get_next_instruction_name(), ins=[], outs=[])
                        nop.engine = inst.engine
                        nop.sync_info = mybir.SyncInfo(on_wait=chunk, on_update=[])
                        nc.register_instruction(nop, overwrite=True)
                        new.append(nop)
                    inst.sync_info = mybir.SyncInfo(
                        on_wait=rest, on_update=list(si.on_update) if si.on_update else [])
                new.append(inst)
            bb.instructions = new


def _layernorm_tile(nc, w, x_t, g_bc, b_bc, eps_t, out_r, affine=True):
    """LayerNorm of one [128, DIM] fp32 tile -> bf16 tile (token-major)."""
    BNF = nc.vector.BN_STATS_FMAX
    nsub = DIM // BNF
    stats = w.tile([P, nsub, nc.vector.BN_STATS_DIM], f32, tag="ln_stats")
    xg = x_t[:].rearrange("p (s f) -> p s f", f=BNF)
    for s_ in range(nsub):
        nc.vector.bn_stats(out=stats[:, s_, :], in_=xg[:, s_, :])
    mv = w.tile([P, nc.vector.BN_AGGR_DIM], f32, tag="ln_mv")
    nc.vector.bn_aggr(out=mv, in_=stats)
    rstd = w.tile([P, 1], f32, tag="ln_rstd")
    nc.scalar.activation(out=rstd, in_=mv[:, 1:2], func=AF.Sqrt, bias=eps_t, scale=1.0)
    nc.vector.reciprocal(out=rstd, in_=rstd)
    if affine:
        tmp = w.tile([P, DIM], f32, tag="ln_tmp")
        nc.vector.tensor_scalar(out=tmp, in0=x_t, scalar1=mv[:, 0:1], scalar2=rstd,
                                op0=ALU.subtract, op1=ALU.mult)
        nc.vector.tensor_mul(out=tmp, in0=tmp, in1=g_bc)
        nc.vector.tensor_add(out=out_r, in0=tmp, in1=b_bc)
    else:
        nc.vector.tensor_scalar(out=out_r, in0=x_t, scalar1=mv[:, 0:1], scalar2=rstd,
                                op0=ALU.subtract, op1=ALU.mult)


def build_kernel(ln1_affine=True, ln2_affine=True, b2fin_zero=False, pairs=True):
    nc = bass.Bass(num_devices=N_CORES)
    groups = REPLICA_PAIRS if pairs else [[c] for c in range(N_CORES)]
    gsz = 2 if pairs else 1

    x_own = nc.dram_tensor("x_own", [T_OWN, DIM], f32, kind="ExternalInput")
    m_scale = nc.dram_tensor("m_scale", [1], f32, kind="ExternalInput")
    # [128, kt, d]: row (kt*128+p) of the original [DIM, DIM] weight
    wq_l = nc.dram_tensor("wq_l", [P, D_T, DIM], bf16, kind="ExternalInput")
    wk_l = nc.dram_tensor("wk_l", [P, D_T, DIM], bf16, kind="ExternalInput")
    wv_l = nc.dram_tensor("wv_l", [P, D_T, DIM], bf16, kind="ExternalInput")
    wo_l = nc.dram_tensor("wo_l", [P, D_T, DIM], bf16, kind="ExternalInput")
    ln1_g = nc.dram_tensor("ln1_g", [DIM], f32, kind="ExternalInput")
    ln1_b = nc.dram_tensor("ln1_b", [DIM], f32, kind="ExternalInput")
    ln2_g = nc.dram_tensor("ln2_g", [DIM], f32, kind="ExternalInput")
    ln2_b = nc.dram_tensor("ln2_b", [DIM], f32, kind="ExternalInput")
    # [lvl, 128, half, kt, 2048]: w1[lvl, kt*128+p, half*2048+c]
    w1_l = nc.dram_tensor("w1_l", [LEVELS, P, 2, D_T, HID // 2], f8, kind="ExternalInput")
    # [lvl, 128, half, kt16, 1024]: w2[lvl, half*2048+kt*128+p, d]
    w2_l = nc.dram_tensor("w2_l", [LEVELS, P, 2, H_T // 2, DIM], f8, kind="ExternalInput")
    b1_l = nc.dram_tensor("b1_l", [LEVELS, P, H_T], f32, kind="ExternalInput")
    b2_l = nc.dram_tensor("b2_l", [LEVELS - 1, P, D_T], f32, kind="ExternalInput")
    b2fin = nc.dram_tensor("b2fin", [DIM], f32, kind="ExternalInput")
    maskT = nc.dram_tensor("maskT", [CHUNK, CHUNK], f32, kind="ExternalInput")
    out = nc.dram_tensor("out", [T_OWN, DIM], f32, kind="ExternalOutput")

    with tile.TileContext(nc) as tc, ExitStack() as top:
        consts = top.enter_context(tc.tile_pool(name="consts", bufs=1))
        ident_f = consts.tile([P, P], f32)
        make_identity(nc, ident_f)
        ident = consts.tile([P, P], bf16)
        nc.vector.tensor_copy(out=ident, in_=ident_f)
        eps_t = consts.tile([P, 1], f32)
        nc.vector.memset(eps_t, EPS)
        # combined super-chunk score mask [e, S0(c0)|S0(c1)|S1]:
        # causal for (k0,q0), all-1 for (k0,q1), causal for (k1,q1)
        maskC = consts.tile([CHUNK, 3 * CHUNK], f32)
        nc.sync.dma_start(out=maskC[:, 0:CHUNK], in_=maskT.ap())
        nc.vector.memset(maskC[:, CHUNK:2 * CHUNK], 1.0)
        nc.sync.dma_start(out=maskC[:, 2 * CHUNK:3 * CHUNK], in_=maskT.ap())
        g1 = b1 = g2 = b2 = None
        if ln1_affine:
            g1 = consts.tile([P, DIM], bf16)
            b1 = consts.tile([P, DIM], bf16)
            nc.sync.dma_start(out=g1, in_=ln1_g.ap()[None, :].partition_broadcast(P).opt())
            nc.sync.dma_start(out=b1, in_=ln1_b.ap()[None, :].partition_broadcast(P).opt())
        if ln2_affine:
            g2 = consts.tile([P, DIM], bf16)
            b2 = consts.tile([P, DIM], bf16)
            nc.sync.dma_start(out=g2, in_=ln2_g.ap()[None, :].partition_broadcast(P).opt())
            nc.sync.dma_start(out=b2, in_=ln2_b.ap()[None, :].partition_broadcast(P).opt())
        msc = consts.tile([P, 1], f32)
        nc.sync.dma_start(out=msc, in_=m_scale.ap()[None, :].partition_broadcast(P).opt())

        # persistent across phases: FFN input h2^T (feature-major, fp8)
        persist = top.enter_context(tc.tile_pool(name="persist", bufs=1))
        hT = persist.tile([P, D_T, T_OWN], f8)
        dram = top.enter_context(tc.tile_pool(name="dram", bufs=1, space="DRAM"))
        x2_d = dram.tile([N_OWN_T, P, DIM], bf16)
        mt_d = dram.tile([P, D_T * HD], f32)
        ag_d = dram.tile([gsz, P, D_T * HD], f32)

        # qT / yT persist until the post-collective fixup + wo projection
        mid = ExitStack()
        qyp = mid.enter_context(tc.tile_pool(name="qyp", bufs=1))
        qT_p = qyp.tile([P, D_T, T_OWN], bf16)
        yT_p = qyp.tile([P, D_T, T_OWN], bf16)
        wo_pool = mid.enter_context(tc.tile_pool(name="wo_pool", bufs=1))
        wo_s = wo_pool.tile([P, D_T, DIM], bf16)
        nc.sync.dma_start(out=wo_s, in_=wo_l.ap())

        # ---------------- attention ----------------
        ab = ExitStack()
        wqkv = ab.enter_context(tc.tile_pool(name="wqkv", bufs=1))
        wq_s = wqkv.tile([P, D_T, DIM], bf16)
        wk_s = wqkv.tile([P, D_T, DIM], bf16)
        wv_s = wqkv.tile([P, D_T, DIM], bf16)
        nc.sync.dma_start(out=wq_s, in_=wq_l.ap())
        nc.sync.dma_start(out=wk_s, in_=wk_l.ap())
        nc.sync.dma_start(out=wv_s, in_=wv_l.ap())

        mt_pool = ab.enter_context(tc.tile_pool(name="mt", bufs=1))
        Mt_f = mt_pool.tile([P, D_T, HD], f32)   # partitions = d of head pair
        Mt_s = mt_pool.tile([P, D_T, HD], bf16)
        nc.vector.memset(Mt_f, 0.0)
        nc.vector.memset(Mt_s, 0.0)

        ln_w = ab.enter_context(tc.tile_pool(name="ln_w", bufs=2))
        xp = ab.enter_context(tc.tile_pool(name="xp", bufs=2))
        hrp = ab.enter_context(tc.tile_pool(name="hrp", bufs=1))
        h1Tp = ab.enter_context(tc.tile_pool(name="h1Tp", bufs=2))
        kvp = ab.enter_context(tc.tile_pool(name="kvp", bufs=2))
        vp = ab.enter_context(tc.tile_pool(name="vp", bufs=1))
        scw = ab.enter_context(tc.tile_pool(name="scw", bufs=2))
        # PSUM budget (8 banks): tp 2 (transposes incl. kc) + mm 4 (proj +
        # scores + fixup, all [P,512] f32 sharing one tag) + ymt 2
        ps_tp = ab.enter_context(tc.tile_pool(name="ps_tp", bufs=2, space="PSUM"))
        ps_mm = ab.enter_context(tc.tile_pool(name="ps_mm", bufs=4, space="PSUM"))
        ps_ymt = ab.enter_context(tc.tile_pool(name="ps_ymt", bufs=2, space="PSUM"))

        for blk in range(N_BLK):
            tok0 = blk * BLK
            # LayerNorm1 + transpose into feature-major h1T
            h1T = h1Tp.tile([P, D_T, BLK], bf16, tag="h1T")
            for t in range(NTT):
                x_t = xp.tile([P, DIM], f32, tag="x")
                nc.sync.dma_start(out=x_t, in_=x_own.ap()[tok0 + t * P:tok0 + (t + 1) * P, :])
                h_r = hrp.tile([P, DIM], bf16, tag="h1r")
                _layernorm_tile(nc, ln_w, x_t, g1, b1, eps_t, h_r, affine=ln1_affine)
                for fp in range(D_T // 2):
                    tps = ps_tp.tile([P, 2, P], bf16, tag="tp")
                    for j in range(2):
                        fi = 2 * fp + j
                        nc.tensor.transpose(tps[:, j, :], h_r[:, fi * P:(fi + 1) * P], ident)
                    nc.vector.tensor_copy(out=h1T[:, 2 * fp:2 * fp + 2, t * P:(t + 1) * P],
                                          in_=tps)
            # q, k: feature-major [feat, tok]
            kT = kvp.tile([P, D_T, BLK], bf16, tag="kT")
            for (w_s, dst, off) in ((wq_s, qT_p, tok0), (wk_s, kT, 0)):
                for m in range(D_T):
                    pst = ps_mm.tile([P, BLK], f32, tag="pst")
                    for k in range(D_T):
                        nc.tensor.matmul(pst, w_s[:, k, m * P:(m + 1) * P], h1T[:, k, :],
                                         start=(k == 0), stop=(k == D_T - 1))
                    nc.scalar.copy(out=dst[:, m, off:off + BLK], in_=pst)
            # v: token-major [tok, feat]
            v = vp.tile([P, NTT, DIM], bf16, tag="v")
            for ti in range(NTT):
                for nh in range(2):
                    pst = ps_mm.tile([P, 512], f32, tag="pst")
                    for k in range(D_T):
                        nc.tensor.matmul(pst, h1T[:, k, ti * P:(ti + 1) * P],
                                         wv_s[:, k, nh * 512:(nh + 1) * 512],
                                         start=(k == 0), stop=(k == D_T - 1))
                    nc.scalar.copy(out=v[:, ti, nh * 512:(nh + 1) * 512], in_=pst)
            # scan: super-chunks of 256 tokens, head-pair packed, feature-major y
            for sch in range(NTT // 2):
                s0 = sch * 2 * P          # in-block offset of chunk c0
                q0 = tok0 + s0            # global offset
                for fi in range(D_T):
                    h0c = (2 * fi) * HD   # head col offsets in token-major v
                    h1c = (2 * fi + 1) * HD
                    # k chunk transposes (both heads at once: [tok, d-pair])
                    kc_ps = ps_tp.tile([P, 2, P], bf16, tag="tp")
                    nc.tensor.transpose(kc_ps[:, 0, :], kT[:, fi, s0:s0 + P], ident)
                    nc.tensor.transpose(kc_ps[:, 1, :], kT[:, fi, s0 + P:s0 + 2 * P], ident)
                    kc = scw.tile([P, 2, P], bf16, tag="kc")
                    nc.scalar.copy(out=kc, in_=kc_ps)
                    # scores [S0(256)|S1(128)|pad] per head (K=64 -> row-group
                    # packed pair), one [P,512] bank each
                    s_ps = [ps_mm.tile([P, 512], f32, tag="pst",
                                       name=f"s_{blk}_{sch}_{fi}_{hh}")
                            for hh in range(2)]
                    for hh in range(2):
                        pb = hh * HD
                        nc.tensor.matmul(s_ps[hh][:, 0:2 * P],
                                         kT[pb:pb + HD, fi, s0:s0 + P],
                                         qT_p[pb:pb + HD, fi, q0:q0 + 2 * P],
                                         start=True, stop=True)
                        nc.tensor.matmul(s_ps[hh][:, 2 * P:3 * P],
                                         kT[pb:pb + HD, fi, s0 + P:s0 + 2 * P],
                                         qT_p[pb:pb + HD, fi, q0 + P:q0 + 2 * P],
                                         start=True, stop=True)
                    s_r = scw.tile([P, 2, 3 * P], bf16, tag="s_r")
                    nc.vector.tensor_mul(out=s_r[:, 0, :], in0=s_ps[0][:, 0:3 * P], in1=maskC)
                    nc.vector.tensor_mul(out=s_r[:, 1, :], in0=s_ps[1][:, 0:3 * P], in1=maskC)
                    # y^T accumulation (intra col-packed + memory diag-packed)
                    # and M update (col-packed), sharing one PSUM bank
                    ymt = ps_ymt.tile([P, 2 * P + HD], f32, tag="ymt")
                    y_ps = ymt[:, 0:2 * P]
                    mt_ps = ymt[:, 2 * P:2 * P + HD]
                    nc.tensor.matmul(y_ps[0:HD, :], v[:, sch * 2, h0c:h0c + HD],
                                     s_r[:, 0, 0:2 * P], start=True, stop=False)
                    nc.tensor.matmul(y_ps[HD:P, :], v[:, sch * 2, h1c:h1c + HD],
                                     s_r[:, 1, 0:2 * P], start=True, stop=False)
                    nc.tensor.matmul(y_ps[0:HD, P:2 * P], v[:, sch * 2 + 1, h0c:h0c + HD],
                                     s_r[:, 0, 2 * P:3 * P], start=False, stop=False)
                    nc.tensor.matmul(y_ps[HD:P, P:2 * P], v[:, sch * 2 + 1, h1c:h1c + HD],
                                     s_r[:, 1, 2 * P:3 * P], start=False, stop=False)
                    nc.tensor.matmul(y_ps[0:HD, :], Mt_s[0:HD, fi, :],
                                     qT_p[0:HD, fi, q0:q0 + 2 * P], start=False, stop=True)
                    nc.tensor.matmul(y_ps[HD:P, :], Mt_s[HD:P, fi, :],
                                     qT_p[HD:P, fi, q0:q0 + 2 * P], start=False, stop=True)
                    nc.scalar.copy(out=yT_p[:, fi, q0:q0 + 2 * P], in_=y_ps)
                    # M update (col-packed pair per chunk)
                    nc.tensor.matmul(mt_ps[0:HD, :], kc[:, 0, 0:HD],
                                     v[:, sch * 2, h0c:h0c + HD], start=True, stop=False)
                    nc.tensor.matmul(mt_ps[HD:P, :], kc[:, 0, HD:P],
                                     v[:, sch * 2, h1c:h1c + HD], start=True, stop=False)
                    nc.tensor.matmul(mt_ps[0:HD, :], kc[:, 1, 0:HD],
                                     v[:, sch * 2 + 1, h0c:h0c + HD], start=False, stop=True)
                    nc.tensor.matmul(mt_ps[HD:P, :], kc[:, 1, HD:P],
                                     v[:, sch * 2 + 1, h1c:h1c + HD], start=False, stop=True)
                    nc.vector.tensor_add(out=Mt_f[:, fi, :], in0=Mt_f[:, fi, :], in1=mt_ps)
                    nc.vector.tensor_copy(out=Mt_s[:, fi, :], in_=Mt_f[:, fi, :])

        # ---------------- pairwise memory-state exchange ----------------
        nc.gpsimd.dma_start(mt_d[:], Mt_f[:].rearrange("p a b -> p (a b)"))
        nc.gpsimd.collective_compute(
            "AllGather",
            mybir.AluOpType.bypass,
            replica_groups=groups,
            ins=[mt_d.opt()],
            outs=[ag_d.opt()],
        )
        mrem_f = mt_pool.tile([P, D_T, HD], f32)
        nc.gpsimd.dma_start(mrem_f[:].rearrange("p a b -> p (a b)"), ag_d[0])
        # even cores start the sequence: scale their received M to zero
        nc.vector.tensor_scalar_mul(out=mrem_f, in0=mrem_f, scalar1=msc[:, 0:1])
        mrem = mt_pool.tile([P, D_T, HD], bf16)
        nc.vector.tensor_copy(out=mrem, in_=mrem_f)
        # y += q @ M_remote over all own tokens
        for fi in range(D_T):
            for g in range(T_OWN // 512):
                ps = ps_mm.tile([P, 512], f32, tag="pst")
                nc.tensor.matmul(ps[0:HD, :], mrem[0:HD, fi, :],
                                 qT_p[0:HD, fi, g * 512:(g + 1) * 512],
                                 start=True, stop=True)
                nc.tensor.matmul(ps[HD:P, :], mrem[HD:P, fi, :],
                                 qT_p[HD:P, fi, g * 512:(g + 1) * 512],
                                 start=True, stop=True)
                nc.vector.tensor_add(out=yT_p[:, fi, g * 512:(g + 1) * 512],
                                     in0=yT_p[:, fi, g * 512:(g + 1) * 512], in1=ps)

        ab.close()

        # ---------------- epilogue: wo, residual, LN2, h2^T ----------------
        ep = ExitStack()
        epw = ep.enter_context(tc.tile_pool(name="epw", bufs=2))
        ep1 = ep.enter_context(tc.tile_pool(name="ep1", bufs=1))
        ln_w2 = ep.enter_context(tc.tile_pool(name="ln_w2", bufs=2))
        ps_wo = ep.enter_context(tc.tile_pool(name="ps_wo", bufs=4, space="PSUM"))
        ps_t2 = ep.enter_context(tc.tile_pool(name="ps_t2", bufs=4, space="PSUM"))
        b2bc = None
        if not b2fin_zero:
            b2bc = ep1.tile([P, DIM], f32)
            nc.sync.dma_start(out=b2bc, in_=b2fin.ap()[None, :].partition_broadcast(P).opt())
        for ti in range(N_OWN_T):
            x_t = epw.tile([P, DIM], f32, tag="ex")
            nc.sync.dma_start(out=x_t, in_=x_own.ap()[ti * P:(ti + 1) * P, :])
            x2f = epw.tile([P, DIM], f32, tag="x2f")
            for nh in range(2):
                pst = ps_wo.tile([P, 512], f32, tag="wo")
                for k in range(D_T):
                    nc.tensor.matmul(pst, yT_p[:, k, ti * P:(ti + 1) * P],
                                     wo_s[:, k, nh * 512:(nh + 1) * 512],
                                     start=(k == 0), stop=(k == D_T - 1))
                nc.vector.tensor_add(out=x2f[:, nh * 512:(nh + 1) * 512],
                                     in0=x_t[:, nh * 512:(nh + 1) * 512], in1=pst)
            # spill x2 (+ final-level FFN bias, pre-added) for the output residual
            x2b = epw.tile([P, DIM], bf16, tag="x2b")
            if b2fin_zero:
                nc.scalar.copy(out=x2b, in_=x2f)
            else:
                nc.vector.tensor_add(out=x2b, in0=x2f, in1=b2bc)
            nc.scalar.dma_start(out=x2_d[ti], in_=x2b)
            h2_r = epw.tile([P, DIM], bf16, tag="h2r")
            _layernorm_tile(nc, ln_w2, x2f, g2, b2, eps_t, h2_r, affine=ln2_affine)
            for fi in range(D_T):
                tps = ps_t2.tile([P, P], bf16, tag="tp2")
                nc.tensor.transpose(tps, h2_r[:, fi * P:(fi + 1) * P], ident)
                nc.scalar.copy(out=hT[:, fi, ti * P:(ti + 1) * P], in_=tps)
        ep.close()
        mid.close()

        # ---------------- CMS FFN ----------------
        n_tt = T_OWN // TT
        with ExitStack() as ffn:
            w1s = ffn.enter_context(tc.tile_pool(name="w1s", bufs=1))
            w2s = ffn.enter_context(tc.tile_pool(name="w2s", bufs=1))
            bp = ffn.enter_context(tc.tile_pool(name="ffn_b", bufs=2))
            big = ffn.enter_context(tc.tile_pool(name="ffn_big", bufs=1))
            upg = big.tile([P, H_T, T_OWN], f8)     # gelu acts, full hidden
            ow = ffn.enter_context(tc.tile_pool(name="ow", bufs=3))
            ps_up = ffn.enter_context(tc.tile_pool(name="ps_up", bufs=4, space="PSUM"))
            ps_dn = ffn.enter_context(tc.tile_pool(name="ps_dn", bufs=4, space="PSUM"))

            def load_level(lvl):
                w1_t = w1s.tile([P, 2, D_T, HID // 2], f8, tag="w1t")
                for half in range(2):
                    nc.sync.dma_start(out=w1_t[:, half], in_=w1_l.ap()[lvl, :, half])
                w2_t = w2s.tile([P, H_T, DIM], f8, tag="w2t")
                for half in range(2):
                    nc.sync.dma_start(
                        out=w2_t[:, half * (H_T // 2):(half + 1) * (H_T // 2), :],
                        in_=w2_l.ap()[lvl, :, half])
                b1_t = bp.tile([P, H_T], f32, tag="b1")
                nc.sync.dma_start(out=b1_t, in_=b1_l.ap()[lvl])
                b2_t = None
                if lvl < LEVELS - 1:
                    b2_t = bp.tile([P, D_T], f32, tag="b2")
                    nc.sync.dma_start(out=b2_t, in_=b2_l.ap()[lvl])
                return w1_t, w2_t, b1_t, b2_t

            for lvl in range(LEVELS):
                w1_t, w2_t, b1_t, b2_t = load_level(lvl)
                # up: h @ w1 -> gelu (feature-major hidden)
                for mh in range(H_T):
                    half, ml = mh // (H_T // 2), mh % (H_T // 2)
                    w1_lhs = w1_t[:, half]
                    for tt in range(n_tt):
                        psl = ps_up.tile([P, TT], f32, tag="up", name=f"up_{lvl}_{mh}_{tt}")
                        for kk in range(D_T // 2):
                            nc.tensor.matmul(
                                psl, w1_lhs[:, 2 * kk:2 * kk + 2, ml * P:(ml + 1) * P],
                                hT[:, 2 * kk:2 * kk + 2, tt * TT:(tt + 1) * TT],
                                start=(kk == 0), stop=(kk == D_T // 2 - 1),
                                perf_mode=PM.DoubleRow)
                        nc.scalar.activation(
                            out=upg[:, mh, tt * TT:(tt + 1) * TT], in_=psl,
                            func=AF.Gelu_apprx_tanh,
                            bias=b1_t[:, mh:mh + 1], scale=INV_W8)
                if lvl < LEVELS - 1:
                    # down, feature-major back into hT (full-hidden PSUM group)
                    for md in range(D_T):
                        for tt in range(n_tt):
                            psl = ps_dn.tile([P, TT], f32, tag="dn", name=f"dn_{lvl}_{md}_{tt}")
                            for kk in range(H_T // 2):
                                nc.tensor.matmul(
                                    psl, w2_t[:, 2 * kk:2 * kk + 2, md * P:(md + 1) * P],
                                    upg[:, 2 * kk:2 * kk + 2, tt * TT:(tt + 1) * TT],
                                    start=(kk == 0), stop=(kk == H_T // 2 - 1),
                                    perf_mode=PM.DoubleRow)
                            nc.scalar.activation(
                                out=hT[:, md, tt * TT:(tt + 1) * TT], in_=psl,
                                func=AF.Identity, bias=b2_t[:, md:md + 1], scale=INV_W8)
                else:
                    # last level: token-major output, fused residual + store
                    for ti in range(N_OWN_T):
                        x2_t = ow.tile([P, DIM], bf16, tag="ox2")
                        nc.sync.dma_start(out=x2_t, in_=x2_d[ti])
                        o_t = ow.tile([P, DIM], f32, tag="oo")
                        for nh in range(2):
                            psl = ps_dn.tile([P, 512], f32, tag="dn", name=f"fin_{ti}_{nh}")
                            for kk in range(H_T // 2):
                                nc.tensor.matmul(
                                    psl, upg[:, 2 * kk:2 * kk + 2, ti * P:(ti + 1) * P],
                                    w2_t[:, 2 * kk:2 * kk + 2, nh * 512:(nh + 1) * 512],
                                    start=(kk == 0), stop=(kk == H_T // 2 - 1),
                                    perf_mode=PM.DoubleRow)
                            htmp = ow.tile([P, 512], bf16, tag="oh")
                            nc.scalar.activation(out=htmp, in_=psl, func=AF.Identity,
                                                 bias=0.0, scale=INV_W8)
                            nc.vector.tensor_add(out=o_t[:, nh * 512:(nh + 1) * 512],
                                                 in0=x2_t[:, nh * 512:(nh + 1) * 512],
                                                 in1=htmp)
                        nc.scalar.dma_start(out=out.ap()[ti * P:(ti + 1) * P, :], in_=o_t)

    _split_multi_waits(nc)
    return nc


_NC_CACHE = {}
LAST_RESULT = None


def _get_nc(key, **kw):
    if key not in _NC_CACHE:
        _NC_CACHE[key] = build_kernel(**kw)
    return _NC_CACHE[key]


def kernel(x, ln1_g, ln1_b, wq, wk, wv, wo, ln2_g, ln2_b,
           cms_w1, cms_b1, cms_w2, cms_b2, **extra):
    import ml_dtypes
    bf = ml_dtypes.bfloat16
    f8h = ml_dtypes.float8_e4m3
    x = np.asarray(x, np.float32)
    maskT = np.triu(np.ones((CHUNK, CHUNK), np.float32))  # maskT[e,c] = e<=c

    def wlin(w):  # [DIM, DIM] -> [128, kt, DIM]
        return np.ascontiguousarray(
            np.asarray(w, np.float32).reshape(D_T, P, DIM).transpose(1, 0, 2).astype(bf))

    w1s = (np.asarray(cms_w1, np.float32) * W8_SCALE)
    w1_h = np.ascontiguousarray(
        w1s.reshape(LEVELS, D_T, P, 2, HID // 2).transpose(0, 2, 3, 1, 4).astype(f8h))
    w2s = (np.asarray(cms_w2, np.float32) * W8_SCALE)
    w2_h = np.ascontiguousarray(
        w2s.reshape(LEVELS, 2, H_T // 2, P, DIM).transpose(0, 3, 1, 2, 4).astype(f8h))
    b1_h = np.ascontiguousarray(
        np.asarray(cms_b1, np.float32).reshape(LEVELS, H_T, P).transpose(0, 2, 1))
    b2a = np.asarray(cms_b2, np.float32)
    b2_h = np.ascontiguousarray(b2a[:LEVELS - 1].reshape(LEVELS - 1, D_T, P).transpose(0, 2, 1))
    b2fin = np.ascontiguousarray(b2a[LEVELS - 1])

    ln1_g = np.asarray(ln1_g, np.float32)
    ln1_b = np.asarray(ln1_b, np.float32)
    ln2_g = np.asarray(ln2_g, np.float32)
    ln2_b = np.asarray(ln2_b, np.float32)
    ln1_affine = not (np.all(ln1_g == 1.0) and np.all(ln1_b == 0.0))
    ln2_affine = not (np.all(ln2_g == 1.0) and np.all(ln2_b == 0.0))
    b2fin_zero = bool(np.all(b2fin == 0.0))

    common = {
        "wq_l": wlin(wq), "wk_l": wlin(wk), "wv_l": wlin(wv), "wo_l": wlin(wo),
        "ln1_g": ln1_g, "ln1_b": ln1_b, "ln2_g": ln2_g, "ln2_b": ln2_b,
        "w1_l": w1_h, "w2_l": w2_h, "b1_l": b1_h, "b2_l": b2_h, "b2fin": b2fin,
        "maskT": maskT,
    }
    in_maps = []
    for c in range(N_CORES):
        b, half = c // 2, c % 2
        own = x[b, half * T_OWN:(half + 1) * T_OWN]
        in_maps.append({**common, "x_own": np.ascontiguousarray(own),
                        "m_scale": np.array([float(half)], np.float32)})
    nc = _get_nc(("v3", ln1_affine, ln2_affine, b2fin_zero),
                 ln1_affine=ln1_affine, ln2_affine=ln2_affine, b2fin_zero=b2fin_zero)
    res = run_bass_kernel_spmd(nc, in_maps, core_ids=list(range(N_CORES)))
    global LAST_RESULT
    LAST_RESULT = res
    out = np.empty((B, S, DIM), np.float32)
    for c in range(N_CORES):
        b, half = c // 2, c % 2
        out[b, half * T_OWN:(half + 1) * T_OWN] = res.results[c]["out"]
    return out
